# revision 1
# baseline (speedup 1.0000x reference)
"""Distributed Bass kernel for nn_Attention_80908593922315 on 8 TRN2 NeuronCores.

Sharding: head-parallel attention (core h owns head h) + spatial-parallel
conv/proj (core j owns flattened-spatial slice [512j, 512j+512)).

Pipeline per core:
  0. PE warm-up dummies (HAM un-throttle) while inputs stream in; conv
     weights DMA'd in tap groups so the first conv matmul issues early
  1. depthwise 3x3x3 conv on PE (27 accumulating diag-matmuls over a
     zero-padded local slab), + per-channel partial stats via ACT accum_out
  2. AllGather of (y, stats) -> full unnormalized y + global instance-norm
     stats; normalization is folded into the QKV weights/biases
  3. per-rank pipeline: y slab DMA -> v/q/k matmuls -> block-0 attention
     units interleaved so ACT/DVE saturate during the qkv ramp; a dummy
     matmul burst gated on the stats DMA re-warms the PE clock gate
  4. attention: scoresT[m-chunk, n-block] via 2x row-packed K=32 matmuls ->
     exp split across ACT (table exp) and DVE (Schraudolph int16 bit-trick
     -> bf16), software-pipelined with scores two units ahead of the AV
     consumers so both exp engines overlap -> AV accumulation in psum
     [33, 512] x2 col-packed halves (row 32 = sum of exp = denominator)
  5. per-group normalize (quake reciprocal + 1 Newton step) spread one op
     per unit through the stream, AllToAll (head-major -> spatial slices),
     1x1 proj on the local 512 columns.
"""

import sys

if "/opt/trn_rl_repo" not in sys.path:
    sys.path.insert(0, "/opt/trn_rl_repo")

import numpy as np
import ml_dtypes

import concourse.bass as bass
import concourse.bacc as bacc
import concourse.tile as tile
import concourse.mybir as mybir
from concourse import bass_utils

BF16 = mybir.dt.bfloat16
F32 = mybir.dt.float32
I16 = mybir.dt.int16
I32 = mybir.dt.int32
AF = mybir.ActivationFunctionType
ALU = mybir.AluOpType
bf16 = ml_dtypes.bfloat16

NCORES = 8
C = 256
NH = 8
HD = 32
HWD = 16
N = HWD * HWD * HWD  # 4096
NS = N // NCORES  # 512 spatial per core
P = 128
CCH = C // P  # 2 channel chunks
EPS = 1e-5
SCALE = HD ** -0.5
NMB = N // P  # 32 m-chunks
NNB = N // 512  # 8 n-blocks
TAPS = [(a, b, c) for a in range(3) for b in range(3) for c in range(3)]

# Schraudolph bf16 exp: exp(x) ~= bitcast_bf16(int16(A*x + B))
SCH_A = 128.0 / float(np.log(2.0))
SCH_B = 16255.5
# quake reciprocal seed: r0 = bitcast(MAGIC - i) = bitcast((i ^ -1) + MAGIC+1)
RECIP_MAGIC = 0x7EF311C3
# s-iterations handled by DVE (rest go to ACT); 7/16 on DVE balances the
# extra per-block combine/normalize work that also lands on DVE
DVE_S = frozenset((1, 3, 5, 7, 9, 11, 13))

_cache = {}


def _build_graph():
    nc = bacc.Bacc(
        "TRN2",
        target_bir_lowering=False,
        debug=False,
        enable_asserts=True,
        num_devices=NCORES,
    )

    # ---- I/O declarations (per-core shards) ----
    x_pad = nc.dram_tensor("x_pad", [CCH, P, 4 * 18 * 18], BF16, kind="ExternalInput").ap()
    dw_diag = nc.dram_tensor("dw_diag", [CCH, 27, P, P], BF16, kind="ExternalInput").ap()
    b_dw = nc.dram_tensor("b_dw", [CCH, P, 1], F32, kind="ExternalInput").ap()
    wq_d = nc.dram_tensor("wq", [CCH, P, P], BF16, kind="ExternalInput").ap()
    wk_d = nc.dram_tensor("wk", [CCH, P, P], BF16, kind="ExternalInput").ap()
    wv_d = nc.dram_tensor("wv", [CCH, P, HD], BF16, kind="ExternalInput").ap()
    bq_d = nc.dram_tensor("bq", [P, 1], F32, kind="ExternalInput").ap()
    bk_d = nc.dram_tensor("bk", [P, 1], F32, kind="ExternalInput").ap()
    bv_d = nc.dram_tensor("bv", [HD, 1], F32, kind="ExternalInput").ap()
    wproj_d = nc.dram_tensor("wproj", [CCH, P, C], BF16, kind="ExternalInput").ap()
    bproj_d = nc.dram_tensor("bproj", [CCH, P, 1], F32, kind="ExternalInput").ap()
    out_d = nc.dram_tensor("out", [CCH, P, NS], F32, kind="ExternalOutput").ap()

    with tile.TileContext(nc) as tc:
        with tc.tile_pool(name="persist", bufs=1) as persist, \
             tc.tile_pool(name="dram", bufs=1, space="DRAM") as dram, \
             tc.tile_pool(name="work", bufs=4) as work:

            # ---- PE warm-up: keep HAM at K=8/8 from the start so the conv
            # matmuls run at 2.4 GHz. ~140 N=128 dummy matmuls cover the
            # input-DMA window (~12us) with sustained PE activity.
            junk = persist.tile([P, P], BF16, name="junk")
            nc.vector.memset(junk[:], 0.0)

            # ---- input DMAs, conv-critical first, spread across queues ----
            xp_sb = persist.tile([P, CCH, 4 * 18 * 18], BF16, name="xp_sb")
            dwd_sb = persist.tile([P, CCH, 27, P], BF16, name="dwd_sb")
            # sync queue: chunk-0 conv inputs (first matmul gate)
            nc.sync.dma_start(xp_sb[:, 0], x_pad[0])
            for g in range(4):
                tg = slice(7 * g, min(27, 7 * g + 7))
                nc.sync.dma_start(
                    dwd_sb[:, 0, tg], dw_diag[0, tg].rearrange("t p q -> p t q")
                )
            # scalar queue: chunk-1 conv inputs (ACT is idle this early)
            nc.scalar.dma_start(xp_sb[:, 1], x_pad[1])
            for g in range(4):
                tg = slice(7 * g, min(27, 7 * g + 7))
                nc.scalar.dma_start(
                    dwd_sb[:, 1, tg], dw_diag[1, tg].rearrange("t p q -> p t q")
                )
            bdw_sb = persist.tile([P, CCH], F32, name="bdw_sb")
            for cc in range(CCH):
                nc.scalar.dma_start(bdw_sb[:, cc : cc + 1], b_dw[cc])
            # gpsimd queue: qkv weights (needed right after the AllGather)
            wq_sb = persist.tile([P, CCH, P], BF16, name="wq_sb")
            wk_sb = persist.tile([P, CCH, P], BF16, name="wk_sb")
            wv_sb = persist.tile([P, CCH, HD], BF16, name="wv_sb")
            for cc in range(CCH):
                nc.gpsimd.dma_start(wq_sb[:, cc], wq_d[cc])
                nc.gpsimd.dma_start(wk_sb[:, cc], wk_d[cc])
                nc.gpsimd.dma_start(wv_sb[:, cc], wv_d[cc])
            bq_sb = persist.tile([P, 1], F32, name="bq_sb")
            bk_sb = persist.tile([P, 1], F32, name="bk_sb")
            bv_sb = persist.tile([HD, 1], F32, name="bv_sb")
            nc.gpsimd.dma_start(bq_sb[:], bq_d)
            nc.gpsimd.dma_start(bk_sb[:], bk_d)
            nc.gpsimd.dma_start(bv_sb[:], bv_d)
            wproj_sb = persist.tile([P, CCH, C], BF16, name="wproj_sb")
            bproj_sb = persist.tile([P, CCH], F32, name="bproj_sb")
            for cc in range(CCH):
                nc.gpsimd.dma_start(wproj_sb[:, cc], wproj_d[cc])
                nc.gpsimd.dma_start(bproj_sb[:, cc : cc + 1], bproj_d[cc])

            # preload the exp activation table early (ACT idle anyway)
            exp_dummy = work.tile([1, 16], BF16, name="exp_dummy")
            nc.vector.memset(exp_dummy[:], 0.0)
            nc.scalar.activation(exp_dummy, exp_dummy, AF.Exp)

            # v ones-columns (softmax denominator rows) set up early
            v_sb = persist.tile([P, NMB * (HD + 1)], BF16, name="v_sb")
            nc.vector.memset(v_sb[:], 1.0)

            # ---- phase 1+2: depthwise conv, then one AllGather of (y, stats)
            y_sb = persist.tile([P, CCH, 514], BF16, name="y_sb")
            stats_sb = persist.tile([P, CCH, 2], F32, name="stats_sb")
            sq_junk = work.tile([P, NS], BF16, name="sq_junk")
            ag_in = dram.tile([P, CCH * 514], BF16, name="ag_in")
            ag_out = dram.tile([NCORES, P, CCH * 514], BF16, name="ag_out",
                               addr_space="Shared")
            with tc.tile_pool(name="warm_psum", bufs=1, space="PSUM") as warm_psum, \
                 tc.tile_pool(name="conv_psum", bufs=2, space="PSUM") as conv_psum:
                wps = warm_psum.tile([1, P], F32, name="wps")
                for _ in range(100):
                    nc.tensor.matmul(wps, lhsT=junk[:, 0:1], rhs=junk[:],
                                     start=True, stop=True)
                for cc in range(CCH):
                    ps = conv_psum.tile([P, NS], F32, name="ps_conv")
                    x4 = xp_sb[:, cc].rearrange("p (a b c) -> p a b c", b=18, c=18)
                    for t, (dh, dw_, dd) in enumerate(TAPS):
                        nc.tensor.matmul(
                            ps,
                            lhsT=dwd_sb[:, cc, t],
                            rhs=x4[:, dh : dh + 2, dw_ : dw_ + 16, dd : dd + 16],
                            start=(t == 0),
                            stop=(t == 26),
                        )
                    nc.scalar.activation(
                        y_sb[:, cc, 0:NS], ps, AF.Identity,
                        bias=bdw_sb[:, cc : cc + 1], scale=1.0,
                        accum_out=stats_sb[:, cc, 0:1],
                    )
                    nc.scalar.activation(
                        sq_junk, ps, AF.Square,
                        bias=bdw_sb[:, cc : cc + 1], scale=1.0,
                        accum_out=stats_sb[:, cc, 1:2],
                    )
                    nc.vector.tensor_copy(y_sb[:, cc, NS : NS + 2], stats_sb[:, cc])
                    # per-chunk bounce, y first (its transfer hides under the
                    # stats chain), then the 2 stats columns gate the trigger
                    q = nc.sync if cc == 0 else nc.gpsimd
                    q.dma_start(
                        ag_in[:, cc * 514 : cc * 514 + NS], y_sb[:, cc, 0:NS]
                    )
                    q.dma_start(
                        ag_in[:, cc * 514 + NS : (cc + 1) * 514],
                        y_sb[:, cc, NS : NS + 2],
                    )
            nc.gpsimd.collective_compute(
                "AllGather",
                ALU.bypass,
                replica_groups=[list(range(NCORES))],
                ins=[ag_in[:].opt()],
                outs=[ag_out[:].opt()],
            )

            y_full = persist.tile([P, CCH, N], BF16, name="y_full")
            stats_g = work.tile([P, CCH, NCORES, 2], BF16, name="stats_g")
            ssum = work.tile([P, CCH, 2], F32, name="ssum")
            ago = ag_out[:].rearrange("r p (q f) -> p q r f", q=CCH)  # [128,2,8,514]
            # stats first (tiny) so the weight-fold chain runs during the
            # 2MB y_full loads instead of queueing behind them
            for cc in range(CCH):
                nc.sync.dma_start(stats_g[:, cc], ago[:, cc, :, NS : NS + 2])
            for cc in range(CCH):
                nc.vector.reduce_sum(
                    ssum[:, cc],
                    stats_g[:, cc].rearrange("p r s -> p s r"),
                    axis=mybir.AxisListType.X,
                )

            mu = persist.tile([P, CCH], F32, name="mu")
            rstd = persist.tile([P, CCH], F32, name="rstd")
            t_a = work.tile([P, 1], F32, name="t_a")
            t_b = work.tile([P, 1], F32, name="t_b")
            t_c = work.tile([P, 1], F32, name="t_c")
            for cc in range(CCH):
                mcc = mu[:, cc : cc + 1]
                rcc = rstd[:, cc : cc + 1]
                nc.vector.tensor_scalar_mul(mcc, ssum[:, cc, 0:1], 1.0 / N)
                nc.vector.tensor_scalar_mul(t_a, ssum[:, cc, 1:2], 1.0 / N)  # E[y^2]
                nc.vector.tensor_tensor(t_b, mcc, mcc, ALU.mult)  # mu^2
                nc.vector.tensor_tensor(t_a, t_a, t_b, ALU.subtract)  # var
                nc.vector.tensor_scalar_add(t_a, t_a, EPS)  # var+eps
                # quake rsqrt seed (no ACT table set needed) + 2 Newton steps
                rci = rcc.bitcast(I32)
                nc.vector.tensor_scalar(
                    rci, t_a.bitcast(I32), 1, -1,
                    op0=ALU.arith_shift_right, op1=ALU.bitwise_xor,
                )
                nc.vector.tensor_scalar_add(rci, rci, 0x5F3759DF + 1)
                for _ in range(2):
                    nc.vector.tensor_tensor(t_b, rcc, rcc, ALU.mult)
                    nc.vector.tensor_tensor(t_c, t_a, t_b, ALU.mult)
                    nc.vector.tensor_scalar(
                        t_c, t_c, -0.5, 1.5, op0=ALU.mult, op1=ALU.add
                    )
                    nc.vector.tensor_tensor(rcc, rcc, t_c, ALU.mult)

            # fold norm scale into qkv weights
            wq_s = persist.tile([P, CCH, P], BF16, name="wq_s")
            wk_s = persist.tile([P, CCH, P], BF16, name="wk_s")
            wv_s = persist.tile([P, CCH, HD], BF16, name="wv_s")
            mu_bf = work.tile([P, CCH], BF16, name="mu_bf")
            nc.vector.tensor_copy(mu_bf[:], mu[:])
            for cc in range(CCH):
                nc.vector.tensor_scalar_mul(wv_s[:, cc], wv_sb[:, cc], rstd[:, cc : cc + 1])
                nc.vector.tensor_scalar_mul(wq_s[:, cc], wq_sb[:, cc], rstd[:, cc : cc + 1])
                nc.vector.tensor_scalar_mul(wk_s[:, cc], wk_sb[:, cc], rstd[:, cc : cc + 1])

            # ---- phases 3+4 interleaved: per-rank qkv + attention ----
            qT = persist.tile([P, N], BF16, name="qT")
            kT = persist.tile([P, N], BF16, name="kT")
            numer4 = persist.tile([P, 2, 512], F32, name="numer4")
            den4 = persist.tile([P, 2, 512], F32, name="den4")
            recip4 = persist.tile([P, 2, 512], F32, name="recip4")
            nc.vector.memset(den4[:], 1.0)
            out4 = persist.tile([P, 2, 512], BF16, name="out4")
            bcast_sb = persist.tile([P, 512], F32, name="bcast_sb")
            t512 = persist.tile([P, 512], F32, name="t512")
            a2a_in = dram.tile([NCORES, HD, NS], BF16, name="a2a_in")
            a2a_out = dram.tile([NCORES, HD, NS], BF16, name="a2a_out")
            bq_eff = persist.tile([P, 1], F32, name="bq_eff")
            bk_eff = persist.tile([P, 1], F32, name="bk_eff")
            bv_eff = persist.tile([HD, 1], F32, name="bv_eff")
            bv4 = persist.tile([P, 1], F32, name="bv4")

            # PSUM layout (8 banks): mm_psum 2x[128,512] + sc_psum 2x[128,1024]
            # + av_psum 2x[97,512] coexist so attention overlaps the qkv ramp.
            with tc.tile_pool(name="mm_psum", bufs=2, space="PSUM") as mm_psum, \
                 tc.tile_pool(name="sc_psum", bufs=2, space="PSUM") as sc_psum, \
                 tc.tile_pool(name="av_psum", bufs=2, space="PSUM") as av_psum, \
                 tc.tile_pool(name="exp_pool", bufs=5) as exp_pool:

                # re-warm the PE right as the AllGather lands: a contiguous
                # dummy-matmul burst (~3.5us) gated on the stats DMA flips the
                # HAM clock gate to 8/8 before the qkv matmuls start
                warm2 = mm_psum.tile([P, 512], F32, name="ps_mm")
                for _ in range(50):
                    nc.tensor.matmul(warm2[0:1, 0:16], lhsT=junk[:, 0:1],
                                     rhs=stats_g[:, 0].rearrange("p r s -> p (r s)"),
                                     start=True, stop=True)

                # effective biases: b' = b - W_scaled @ mu  (q also * SCALE)
                pq = mm_psum.tile([P, 512], F32, name="ps_mm")
                for cc in range(CCH):
                    nc.tensor.matmul(pq[:, 0:1], lhsT=wq_s[:, cc],
                                     rhs=mu_bf[:, cc : cc + 1],
                                     start=(cc == 0), stop=(cc == CCH - 1))
                pk = mm_psum.tile([P, 512], F32, name="ps_mm")
                for cc in range(CCH):
                    nc.tensor.matmul(pk[:, 0:1], lhsT=wk_s[:, cc],
                                     rhs=mu_bf[:, cc : cc + 1],
                                     start=(cc == 0), stop=(cc == CCH - 1))
                pv = mm_psum.tile([P, 512], F32, name="ps_mm")
                for cc in range(CCH):
                    nc.tensor.matmul(pv[0:HD, 0:1], lhsT=wv_s[:, cc],
                                     rhs=mu_bf[:, cc : cc + 1],
                                     start=(cc == 0), stop=(cc == CCH - 1))
                nc.vector.tensor_tensor(bq_eff, bq_sb, pq[:, 0:1], ALU.subtract)
                nc.vector.tensor_scalar_mul(bq_eff, bq_eff, SCALE)
                nc.vector.tensor_tensor(bk_eff, bk_sb, pk[:, 0:1], ALU.subtract)
                nc.vector.tensor_tensor(bv_eff, bv_sb, pv[0:HD, 0:1], ALU.subtract)
                for i in range(4):
                    nc.vector.tensor_copy(bv4[32 * i : 32 * i + 32], bv_eff)

                av_tiles = {}
                ex_tiles = {}

                def attn_score(b, s):
                    # score matmuls + exp; the AV consumer is emitted two
                    # units later (software pipeline) so the critical chain is
                    # exp(i) -> sc(i+2) -> exp(i+2) and the two exp engines
                    # stay saturated
                    if s == 0:
                        av_tiles[b] = av_psum.tile([97, 512], F32, name="av")
                    sc = sc_psum.tile([P, 1024], F32, name="sc")
                    for r in range(2):
                        cm = 2 * s + r
                        nc.tensor.matmul(
                            sc[:, r * 512 : (r + 1) * 512],
                            lhsT=kT[32 * r : 32 * (r + 1), cm * P : (cm + 1) * P],
                            rhs=qT[32 * r : 32 * (r + 1), b * 512 : (b + 1) * 512],
                            start=True, stop=True,
                            tile_position=(32 * r, 0),
                        )
                    ex = exp_pool.tile([P, 1024], BF16, name="ex")
                    if s in DVE_S:
                        # Schraudolph bf16 exp on DVE: one fused mul-add with
                        # int16 convert-on-write, bit-viewed as bf16
                        nc.vector.tensor_scalar(
                            ex.bitcast(I16)[:], sc[:], SCH_A, SCH_B,
                            op0=ALU.mult, op1=ALU.add,
                        )
                    else:
                        nc.scalar.activation(ex, sc[:], AF.Exp)
                    ex_tiles[(b, s)] = ex

                def attn_av(b, s):
                    # two col-packed halves (partitions 0-32 / 64-96) run
                    # concurrently on disjoint PE column strips
                    av = av_tiles[b]
                    ex = ex_tiles.pop((b, s))
                    for r in range(2):
                        cm = 2 * s + r
                        nc.tensor.matmul(
                            av[64 * r : 64 * r + HD + 1, :],
                            lhsT=v_sb[:, cm * (HD + 1) : (cm + 1) * (HD + 1)],
                            rhs=ex[:, r * 512 : (r + 1) * 512],
                            start=(s == 0),
                            stop=(s == NMB // 2 - 1),
                            tile_position=(0, 64 * r),
                            skip_group_check=True,
                        )

                epilogue_q = []

                def attn_block_end(b):
                    # queue the evacuate/combine/normalize pieces; one piece
                    # is emitted per subsequent unit so the DVE/ACT queues
                    # never bulge and stall the exp pipeline
                    av = av_tiles.pop(b)
                    tmp33 = work.tile([HD + 1, 512], F32, name="tmp33")
                    q0 = b % 4
                    g = b // 4
                    epilogue_q.append(lambda: nc.scalar.activation(
                        tmp33, av[64 : 64 + HD + 1, :], AF.Identity))
                    epilogue_q.append(lambda: nc.vector.tensor_tensor(
                        numer4[32 * q0 : 32 * q0 + HD, g, :],
                        av[0:HD, :], tmp33[0:HD, :], ALU.add))
                    epilogue_q.append(lambda: nc.vector.tensor_tensor(
                        den4[32 * q0 : 32 * q0 + 1, g, :],
                        av[HD : HD + 1, :], tmp33[HD : HD + 1, :], ALU.add))
                    if b % 4 == 3:
                        # normalize this group of 4 blocks; quake reciprocal
                        # seed + 1 Newton step stays DVE-only and cheap
                        dg = den4[:, g, :]
                        rg = recip4[:, g, :]
                        epilogue_q.append(lambda: nc.vector.tensor_scalar(
                            rg.bitcast(I32), dg.bitcast(I32), 0, -1,
                            op0=ALU.arith_shift_right, op1=ALU.bitwise_xor))
                        epilogue_q.append(lambda: nc.vector.tensor_scalar_add(
                            rg.bitcast(I32), rg.bitcast(I32), RECIP_MAGIC + 1))
                        epilogue_q.append(lambda: nc.vector.tensor_tensor(
                            t512, dg, rg, ALU.mult))
                        epilogue_q.append(lambda: nc.vector.tensor_scalar(
                            t512, t512, -1.0, 2.0, op0=ALU.mult, op1=ALU.add))
                        epilogue_q.append(lambda: nc.vector.tensor_tensor(
                            rg, rg, t512, ALU.mult))
                        epilogue_q.append(lambda: nc.vector.stream_shuffle(
                            bcast_sb[:], rg, mask=[0] * 32))
                        epilogue_q.append(lambda: nc.vector.tensor_tensor(
                            out4[:, g, :], numer4[:, g, :], bcast_sb, ALU.mult))
                        epilogue_q.append(lambda: nc.vector.tensor_scalar_add(
                            out4[:, g, :], out4[:, g, :], bv4))
                        for j in range(4 * g, 4 * g + 4):
                            epilogue_q.append(
                                lambda j=j: (nc.sync if j % 2 == 0 else nc.gpsimd)
                                .dma_start(
                                    a2a_in[j],
                                    out4[32 * (j % 4) : 32 * (j % 4) + HD, j // 4, :],
                                ))

                def qkv_rank(r):
                    for cc in range(CCH):
                        (nc.sync if r % 2 == 0 else nc.gpsimd).dma_start(
                            y_full[:, cc, r * NS : (r + 1) * NS],
                            ago[:, cc, r, 0:NS],
                        )
                    # v for the rank's 4 m-chunks
                    for cl in range(4):
                        mb = 4 * r + cl
                        psv = mm_psum.tile([P, 512], F32, name="ps_mm")
                        for cc in range(CCH):
                            nc.tensor.matmul(
                                psv[:, 0:HD],
                                lhsT=y_full[:, cc, mb * P : (mb + 1) * P],
                                rhs=wv_s[:, cc],
                                start=(cc == 0), stop=(cc == CCH - 1),
                            )
                        if cl % 2 == 0:
                            nc.vector.tensor_copy(
                                v_sb[:, mb * (HD + 1) : mb * (HD + 1) + HD],
                                psv[:, 0:HD],
                            )
                        else:
                            nc.scalar.activation(
                                v_sb[:, mb * (HD + 1) : mb * (HD + 1) + HD],
                                psv[:, 0:HD], AF.Identity,
                            )
                    psq = mm_psum.tile([P, 512], F32, name="ps_mm")
                    for cc in range(CCH):
                        nc.tensor.matmul(
                            psq, lhsT=wq_s[:, cc],
                            rhs=y_full[:, cc, r * 512 : (r + 1) * 512],
                            start=(cc == 0), stop=(cc == CCH - 1),
                        )
                    nc.scalar.activation(
                        qT[:, r * 512 : (r + 1) * 512], psq, AF.Identity,
                        bias=bq_eff, scale=SCALE,
                    )
                    psk = mm_psum.tile([P, 512], F32, name="ps_mm")
                    for cc in range(CCH):
                        nc.tensor.matmul(
                            psk, lhsT=wk_s[:, cc],
                            rhs=y_full[:, cc, r * 512 : (r + 1) * 512],
                            start=(cc == 0), stop=(cc == CCH - 1),
                        )
                    nc.vector.tensor_scalar_add(
                        kT[:, r * 512 : (r + 1) * 512], psk, bk_eff
                    )

                # unit stream: block-0 units interleave with the per-rank qkv
                # ramp; scores run two units ahead of AVs so the critical
                # chain is exp->sc->exp (the sc-buffer WAR edge) and both exp
                # engines stay saturated
                units = [(b, s) for b in range(NNB) for s in range(NMB // 2)]

                def prework(i):
                    b, s = units[i]
                    if b == 0 and s % 2 == 0:
                        qkv_rank(s // 2)

                for i in range(2):
                    prework(i)
                    attn_score(*units[i])
                for i, u in enumerate(units):
                    if i + 2 < len(units):
                        prework(i + 2)
                        attn_score(*units[i + 2])
                    attn_av(*u)
                    if u[1] == NMB // 2 - 1:
                        attn_block_end(u[0])
                    if epilogue_q:
                        epilogue_q.pop(0)()
                while epilogue_q:
                    epilogue_q.pop(0)()

            # ---- phase 5: all-to-all + projection ----
            nc.gpsimd.collective_compute(
                "AllToAll",
                ALU.bypass,
                replica_groups=[list(range(NCORES))],
                ins=[a2a_in[:].opt()],
                outs=[a2a_out[:].opt()],
            )
            cat = a2a_out[:].rearrange("h d f -> (h d) f")  # [256, 512]
            c_sb = persist.tile([P, CCH, NS], BF16, name="c_sb")
            for cc in range(CCH):
                (nc.sync if cc == 0 else nc.gpsimd).dma_start(
                    c_sb[:, cc], cat[cc * P : (cc + 1) * P]
                )
            out_sb = persist.tile([P, CCH, NS], F32, name="out_sb")
            with tc.tile_pool(name="proj_psum", bufs=2, space="PSUM") as proj_psum:
                for ob in range(CCH):
                    psp = proj_psum.tile([P, NS], F32, name="ps_proj")
                    for cc in range(CCH):
                        nc.tensor.matmul(
                            psp,
                            lhsT=wproj_sb[:, cc, ob * P : (ob + 1) * P],
                            rhs=c_sb[:, cc],
                            start=(cc == 0), stop=(cc == CCH - 1),
                        )
                    nc.scalar.activation(
                        out_sb[:, ob], psp, AF.Identity,
                        bias=bproj_sb[:, ob : ob + 1], scale=1.0,
                    )
                    (nc.sync if ob == 0 else nc.gpsimd).dma_start(
                        out_d[ob], out_sb[:, ob])

    nc.compile()
    return nc


def _host_prep(x, w_dw, b_dw, w_qkv, b_qkv, w_proj, b_proj):
    """Build per-core in_maps from full inputs."""
    x = np.asarray(x, np.float32)
    w_dw = np.asarray(w_dw, np.float32)
    b_dw = np.asarray(b_dw, np.float32)
    w_qkv = np.asarray(w_qkv, np.float32)
    b_qkv = np.asarray(b_qkv, np.float32)
    w_proj = np.asarray(w_proj, np.float32)
    b_proj = np.asarray(b_proj, np.float32)

    xs = x[0]  # [C, 16, 16, 16]
    # diag conv weights: [CCH, 27, P, P]
    dw_diag = np.zeros((CCH, 27, P, P), np.float32)
    for cc in range(CCH):
        for t, (dh, dw_, dd) in enumerate(TAPS):
            np.fill_diagonal(dw_diag[cc, t], w_dw[cc * P : (cc + 1) * P, 0, dh, dw_, dd])
    dw_diag = dw_diag.astype(bf16)
    b_dw_s = b_dw.reshape(CCH, P, 1)

    wproj_t = np.ascontiguousarray(w_proj.T).reshape(CCH, P, C).astype(bf16)
    bproj_s = b_proj.reshape(CCH, P, 1)

    in_maps = []
    for h in range(NCORES):
        # padded x slab: global h rows 2h-1 .. 2h+2, padded w/d
        xp = np.zeros((C, 4, 18, 18), np.float32)
        for hl in range(4):
            hg = 2 * h - 1 + hl
            if 0 <= hg < HWD:
                xp[:, hl, 1:17, 1:17] = xs[:, hg]
        xp = xp.reshape(CCH, P, 4 * 18 * 18).astype(bf16)

        wq_h = w_qkv[h * HD : (h + 1) * HD]  # [32, 256]
        wk_h = w_qkv[C + h * HD : C + (h + 1) * HD]
        wv_h = w_qkv[2 * C + h * HD : 2 * C + (h + 1) * HD]
        wq_rep = np.tile(wq_h.T, (1, 4)).reshape(C, P)  # [256, 128]
        wk_rep = np.tile(wk_h.T, (1, 4)).reshape(C, P)
        in_maps.append({
            "x_pad": xp,
            "dw_diag": dw_diag,
            "b_dw": b_dw_s,
            "wq": wq_rep.reshape(CCH, P, P).astype(bf16),
            "wk": wk_rep.reshape(CCH, P, P).astype(bf16),
            "wv": np.ascontiguousarray(wv_h.T).reshape(CCH, P, HD).astype(bf16),
            "bq": np.tile(b_qkv[h * HD : (h + 1) * HD], 4).reshape(P, 1).astype(np.float32),
            "bk": np.tile(b_qkv[C + h * HD : C + (h + 1) * HD], 4).reshape(P, 1).astype(np.float32),
            "bv": b_qkv[2 * C + h * HD : 2 * C + (h + 1) * HD].reshape(HD, 1).astype(np.float32),
            "wproj": wproj_t,
            "bproj": bproj_s,
        })
    return in_maps


def kernel(**inputs):
    if "nc" not in _cache:
        _cache["nc"] = _build_graph()
    nc = _cache["nc"]
    in_maps = _host_prep(**inputs)
    res = bass_utils.run_bass_kernel_spmd(nc, in_maps, core_ids=list(range(NCORES)))
    slices = [res.results[j]["out"].reshape(C, NS) for j in range(NCORES)]
    full = np.concatenate(slices, axis=1)  # [256, 4096]
    return full.reshape(1, C, HWD, HWD, HWD).astype(np.float32)


if __name__ == "__main__":
    nc = _build_graph()
    print("graph built + compiled OK")



# revision 17
# speedup vs baseline: 1.2144x; 1.2144x over previous
"""Distributed Bass kernel for nn_Attention_80908593922315 on 8 TRN2 NeuronCores.

Sharding: head-parallel attention (core h owns head h) + spatial-parallel
conv/proj (core j owns flattened-spatial slice [512j, 512j+512)).

Pipeline per core:
  0. PE warm-up dummies (HAM un-throttle) while inputs stream in
  1. depthwise 3x3x3 conv on PE (27 accumulating diag-matmuls over a
     zero-padded local slab), + per-channel partial stats via ACT accum_out
  2. AllGather of (y, stats) -> full unnormalized y + global instance-norm
     stats; normalization is folded into the QKV weights/biases
  3. per-rank pipeline: y slab DMA -> v/q/k matmuls interleaved with block-0
     attention units; dummy burst gated on the stats DMA re-warms the PE
  4. attention: per unit (4 key-chunks x 256-query strip) scores via FOUR
     K=32 row-group-packed matmuls (concurrent on PE sub-arrays) -> exp
     split ACT (table exp) / DVE (Schraudolph int16 bit-trick), statically
     load-balanced -> AV accumulation [97, 256-strip] x2 col-packed halves
     (row 32 = sum of exp = denominator); small filler matmuls pad PE gaps
     so the HAM clock gate stays at 8/8
  5. per-group normalize (single-op DVE reciprocal + partition broadcast),
     AllToAll (head-major -> spatial slices), 1x1 proj on the local 512
     columns. k-bias is dropped entirely (softmax shift-invariance).
"""

import sys

if "/opt/trn_rl_repo" not in sys.path:
    sys.path.insert(0, "/opt/trn_rl_repo")

import numpy as np
import ml_dtypes

import concourse.bass as bass
import concourse.bacc as bacc
import concourse.tile as tile
import concourse.mybir as mybir
from concourse import bass_utils

BF16 = mybir.dt.bfloat16
F32 = mybir.dt.float32
I16 = mybir.dt.int16
I32 = mybir.dt.int32
AF = mybir.ActivationFunctionType
ALU = mybir.AluOpType
bf16 = ml_dtypes.bfloat16

NCORES = 8
C = 256
NH = 8
HD = 32
HWD = 16
N = HWD * HWD * HWD  # 4096
NS = N // NCORES  # 512 spatial per core
P = 128
CCH = C // P  # 2 channel chunks
EPS = 1e-5
SCALE = HD ** -0.5
NMB = N // P  # 32 m-chunks
NNB = N // 512  # 8 n-blocks
TAPS = [(a, b, c) for a in range(3) for b in range(3) for c in range(3)]

# Schraudolph bf16 exp: exp(x) ~= bitcast_bf16(int16(A*x + B))
SCH_A = 128.0 / float(np.log(2.0))
SCH_B = 16255.5

# PE filler matmuls per unit (before scores / before AV) to keep the HAM
# clock gate at K=8/8 during exp-bound stretches; [1, FILL_W] each
import os
FILL_SC = int(os.environ.get("K_FILL_SC", "0"))
FILL_AV = int(os.environ.get("K_FILL_AV", "0"))
FILL_W = 96
USE_POOL_NORM = os.environ.get("K_POOL_NORM", "0") == "1"
USE_RECIP_FAST = os.environ.get("K_RECIP_FAST", "0") == "1"
# quake reciprocal seed: r0 = bitcast(MAGIC - i) = bitcast((i ^ -1) + MAGIC+1)
RECIP_MAGIC = 0x7EF311C3
ALL_OLD = os.environ.get("K_ALL_OLD", "1") == "1"
# row-group PE positions for the 4 score matmuls of a new-flavor unit
SC_POS = [int(x) for x in os.environ.get("K_SC_POS", "0,32,64,96").split(",")]

_cache = {}


def _exp_schedule(units):
    """Static greedy ACT/DVE assignment for the per-unit exp op.

    Models per-unit engine-busy in us: ACT tile ~0.996, DVE tile ~1.128,
    plus the side work each engine owes (qkv ramp writes, epilogue).
    """
    A = 0.0
    D = 0.0
    sched = []
    for un in units:
        if un[0] == "old" and un[1] == 0 and un[2] % 2 == 0:
            A += 0.85  # qT write + v-half
            D += 0.65  # kT write + v-half
        if A + 0.996 <= D + 1.128:
            sched.append(True)
            A += 0.996
        else:
            sched.append(False)
            D += 1.128
        b = un[1]
        is_last = (un[2] == 15) if un[0] == "old" else (un[2] == 1 and un[3] == 7)
        if is_last:
            A += 0.6  # block-end tmp33 evacuate
            D += 1.32  # block-end combine adds
            if b % 4 == 3:
                D += 1.3  # recip + shuffle
                if b // 4 == 1:
                    D += 1.2  # group-1 out4 normalize stays on DVE
    return sched


def _build_graph():
    nc = bacc.Bacc(
        "TRN2",
        target_bir_lowering=False,
        debug=False,
        enable_asserts=True,
        num_devices=NCORES,
    )

    # ---- I/O declarations (per-core shards) ----
    x_pad = nc.dram_tensor("x_pad", [CCH, P, 4 * 18 * 18], BF16, kind="ExternalInput").ap()
    dw_diag = nc.dram_tensor("dw_diag", [CCH, 27, P, P], BF16, kind="ExternalInput").ap()
    b_dw = nc.dram_tensor("b_dw", [CCH, P, 1], F32, kind="ExternalInput").ap()
    wq_d = nc.dram_tensor("wq", [CCH, P, P], BF16, kind="ExternalInput").ap()
    wk_d = nc.dram_tensor("wk", [CCH, P, P], BF16, kind="ExternalInput").ap()
    wv_d = nc.dram_tensor("wv", [CCH, P, HD], BF16, kind="ExternalInput").ap()
    bq_d = nc.dram_tensor("bq", [P, 1], F32, kind="ExternalInput").ap()
    bv_d = nc.dram_tensor("bv", [HD, 1], F32, kind="ExternalInput").ap()
    wproj_d = nc.dram_tensor("wproj", [CCH, P, C], BF16, kind="ExternalInput").ap()
    bproj_d = nc.dram_tensor("bproj", [CCH, P, 1], F32, kind="ExternalInput").ap()
    out_d = nc.dram_tensor("out", [CCH, P, NS], F32, kind="ExternalOutput").ap()

    with tile.TileContext(nc) as tc:
        with tc.tile_pool(name="persist", bufs=1) as persist, \
             tc.tile_pool(name="dram", bufs=1, space="DRAM") as dram, \
             tc.tile_pool(name="work", bufs=4) as work:

            # ---- PE warm-up: keep HAM at K=8/8 from the start so the conv
            # matmuls run at 2.4 GHz; covers the input-DMA window.
            junk = persist.tile([P, P], BF16, name="junk")
            nc.vector.memset(junk[:], 0.0)

            # ---- input DMAs, conv-critical first, spread across queues ----
            xp_sb = persist.tile([P, CCH, 4 * 18 * 18], BF16, name="xp_sb")
            dwd_sb = persist.tile([P, CCH, 27, P], BF16, name="dwd_sb")
            # sync queue: chunk-0 conv inputs (first matmul gate)
            nc.sync.dma_start(xp_sb[:, 0], x_pad[0])
            for g in range(4):
                tg = slice(7 * g, min(27, 7 * g + 7))
                nc.sync.dma_start(
                    dwd_sb[:, 0, tg], dw_diag[0, tg].rearrange("t p q -> p t q")
                )
            # scalar queue: chunk-1 conv inputs (ACT is idle this early)
            nc.scalar.dma_start(xp_sb[:, 1], x_pad[1])
            for g in range(4):
                tg = slice(7 * g, min(27, 7 * g + 7))
                nc.scalar.dma_start(
                    dwd_sb[:, 1, tg], dw_diag[1, tg].rearrange("t p q -> p t q")
                )
            bdw_sb = persist.tile([P, CCH], F32, name="bdw_sb")
            for cc in range(CCH):
                nc.scalar.dma_start(bdw_sb[:, cc : cc + 1], b_dw[cc])
            # gpsimd queue: qkv weights (needed right after the AllGather)
            wq_sb = persist.tile([P, CCH, P], BF16, name="wq_sb")
            wk_sb = persist.tile([P, CCH, P], BF16, name="wk_sb")
            wv_sb = persist.tile([P, CCH, HD], BF16, name="wv_sb")
            for cc in range(CCH):
                nc.gpsimd.dma_start(wq_sb[:, cc], wq_d[cc])
                nc.gpsimd.dma_start(wk_sb[:, cc], wk_d[cc])
                nc.gpsimd.dma_start(wv_sb[:, cc], wv_d[cc])
            bq_sb = persist.tile([P, 1], F32, name="bq_sb")
            bv_sb = persist.tile([HD, 1], F32, name="bv_sb")
            nc.gpsimd.dma_start(bq_sb[:], bq_d)
            nc.gpsimd.dma_start(bv_sb[:], bv_d)
            wproj_sb = persist.tile([P, CCH, C], BF16, name="wproj_sb")
            bproj_sb = persist.tile([P, CCH], F32, name="bproj_sb")
            for cc in range(CCH):
                nc.gpsimd.dma_start(wproj_sb[:, cc], wproj_d[cc])
                nc.gpsimd.dma_start(bproj_sb[:, cc : cc + 1], bproj_d[cc])

            # preload the exp activation table early (ACT idle anyway)
            exp_dummy = work.tile([1, 16], BF16, name="exp_dummy")
            nc.vector.memset(exp_dummy[:], 0.0)
            nc.scalar.activation(exp_dummy, exp_dummy, AF.Exp)

            # v ones-columns (softmax denominator rows) set up early
            v_sb = persist.tile([P, NMB * (HD + 1)], BF16, name="v_sb")
            nc.vector.memset(v_sb[:], 1.0)

            # ---- phase 1+2: depthwise conv, then one AllGather of (y, stats)
            y_sb = persist.tile([P, CCH, 514], BF16, name="y_sb")
            stats_sb = persist.tile([P, CCH, 2], F32, name="stats_sb")
            sq_junk = work.tile([P, NS], BF16, name="sq_junk")
            ag_in = dram.tile([P, CCH * 514], BF16, name="ag_in")
            ag_out = dram.tile([NCORES, P, CCH * 514], BF16, name="ag_out",
                               addr_space="Shared")
            with tc.tile_pool(name="warm_psum", bufs=1, space="PSUM") as warm_psum, \
                 tc.tile_pool(name="conv_psum", bufs=2, space="PSUM") as conv_psum:
                wps = warm_psum.tile([1, P], F32, name="wps")
                for _ in range(100):
                    nc.tensor.matmul(wps, lhsT=junk[:, 0:1], rhs=junk[:],
                                     start=True, stop=True)
                for cc in range(CCH):
                    ps = conv_psum.tile([P, NS], F32, name="ps_conv")
                    x4 = xp_sb[:, cc].rearrange("p (a b c) -> p a b c", b=18, c=18)
                    for t, (dh, dw_, dd) in enumerate(TAPS):
                        nc.tensor.matmul(
                            ps,
                            lhsT=dwd_sb[:, cc, t],
                            rhs=x4[:, dh : dh + 2, dw_ : dw_ + 16, dd : dd + 16],
                            start=(t == 0),
                            stop=(t == 26),
                        )
                    nc.scalar.activation(
                        y_sb[:, cc, 0:NS], ps, AF.Identity,
                        bias=bdw_sb[:, cc : cc + 1], scale=1.0,
                        accum_out=stats_sb[:, cc, 0:1],
                    )
                    nc.scalar.activation(
                        sq_junk, ps, AF.Square,
                        bias=bdw_sb[:, cc : cc + 1], scale=1.0,
                        accum_out=stats_sb[:, cc, 1:2],
                    )
                    nc.vector.tensor_copy(y_sb[:, cc, NS : NS + 2], stats_sb[:, cc])
                    # per-chunk bounce, y first (its transfer hides under the
                    # stats chain), then the 2 stats columns gate the trigger
                    q = nc.sync if cc == 0 else nc.scalar
                    q.dma_start(
                        ag_in[:, cc * 514 : cc * 514 + NS], y_sb[:, cc, 0:NS]
                    )
                    q.dma_start(
                        ag_in[:, cc * 514 + NS : (cc + 1) * 514],
                        y_sb[:, cc, NS : NS + 2],
                    )
            nc.gpsimd.collective_compute(
                "AllGather",
                ALU.bypass,
                replica_groups=[list(range(NCORES))],
                ins=[ag_in[:].opt()],
                outs=[ag_out[:].opt()],
            )

            y_full = persist.tile([P, CCH, N], BF16, name="y_full")
            stats_g = work.tile([P, CCH, NCORES, 2], BF16, name="stats_g")
            ssum = work.tile([P, CCH, 2], F32, name="ssum")
            ago = ag_out[:].rearrange("r p (q f) -> p q r f", q=CCH)  # [128,2,8,514]
            # stats first (tiny) so the weight-fold chain runs during the
            # 2MB y_full loads instead of queueing behind them
            for cc in range(CCH):
                nc.sync.dma_start(stats_g[:, cc], ago[:, cc, :, NS : NS + 2])
            for cc in range(CCH):
                nc.vector.reduce_sum(
                    ssum[:, cc],
                    stats_g[:, cc].rearrange("p r s -> p s r"),
                    axis=mybir.AxisListType.X,
                )

            mu = persist.tile([P, CCH], F32, name="mu")
            rstd = persist.tile([P, CCH], F32, name="rstd")
            t_a = work.tile([P, 1], F32, name="t_a")
            t_b = work.tile([P, 1], F32, name="t_b")
            t_c = work.tile([P, 1], F32, name="t_c")
            for cc in range(CCH):
                mcc = mu[:, cc : cc + 1]
                rcc = rstd[:, cc : cc + 1]
                nc.vector.tensor_scalar_mul(mcc, ssum[:, cc, 0:1], 1.0 / N)
                nc.vector.tensor_scalar_mul(t_a, ssum[:, cc, 1:2], 1.0 / N)  # E[y^2]
                nc.vector.tensor_tensor(t_b, mcc, mcc, ALU.mult)  # mu^2
                nc.vector.tensor_tensor(t_a, t_a, t_b, ALU.subtract)  # var
                nc.vector.tensor_scalar_add(t_a, t_a, EPS)  # var+eps
                # quake rsqrt seed (no ACT table set needed) + 2 Newton steps
                rci = rcc.bitcast(I32)
                nc.vector.tensor_scalar(
                    rci, t_a.bitcast(I32), 1, -1,
                    op0=ALU.arith_shift_right, op1=ALU.bitwise_xor,
                )
                nc.vector.tensor_scalar_add(rci, rci, 0x5F3759DF + 1)
                for _ in range(2):
                    nc.vector.tensor_tensor(t_b, rcc, rcc, ALU.mult)
                    nc.vector.tensor_tensor(t_c, t_a, t_b, ALU.mult)
                    nc.vector.tensor_scalar(
                        t_c, t_c, -0.5, 1.5, op0=ALU.mult, op1=ALU.add
                    )
                    nc.vector.tensor_tensor(rcc, rcc, t_c, ALU.mult)

            # fold norm scale into qkv weights
            wq_s = persist.tile([P, CCH, P], BF16, name="wq_s")
            wk_s = persist.tile([P, CCH, P], BF16, name="wk_s")
            wv_s = persist.tile([P, CCH, HD], BF16, name="wv_s")
            mu_bf = work.tile([P, CCH], BF16, name="mu_bf")
            nc.vector.tensor_copy(mu_bf[:], mu[:])
            for cc in range(CCH):
                nc.vector.tensor_scalar_mul(wv_s[:, cc], wv_sb[:, cc], rstd[:, cc : cc + 1])
                nc.vector.tensor_scalar_mul(wq_s[:, cc], wq_sb[:, cc], rstd[:, cc : cc + 1])
                nc.vector.tensor_scalar_mul(wk_s[:, cc], wk_sb[:, cc], rstd[:, cc : cc + 1])

            # ---- phases 3+4 interleaved: per-rank qkv + attention ----
            qT = persist.tile([P, N], BF16, name="qT")
            kT = persist.tile([P, N], BF16, name="kT")
            numer4 = persist.tile([P, 2, 512], F32, name="numer4")
            den4 = persist.tile([P, 2, 512], F32, name="den4")
            recip4 = persist.tile([P, 2, 512], F32, name="recip4")
            bcast4 = persist.tile([P, 2, 512], F32, name="bcast4")
            t512 = persist.tile([P, 2, 512], F32, name="t512")
            nc.vector.memset(den4[:], 1.0)
            out4 = persist.tile([P, 2, 512], BF16, name="out4")
            a2a_in = dram.tile([NCORES, HD, NS], BF16, name="a2a_in")
            a2a_out = dram.tile([NCORES, HD, NS], BF16, name="a2a_out")
            bq_eff = persist.tile([P, 1], F32, name="bq_eff")
            bv_eff = persist.tile([HD, 1], F32, name="bv_eff")
            bv4 = persist.tile([P, 1], F32, name="bv4")

            # PSUM layout (8 banks): mm_psum 2x[128,512] + sc_psum 2x[128,1024]
            # + av_psum 2x[97,512] coexist so attention overlaps the qkv ramp.
            with tc.tile_pool(name="mm_psum", bufs=2, space="PSUM") as mm_psum, \
                 tc.tile_pool(name="sc_psum", bufs=2, space="PSUM") as sc_psum, \
                 tc.tile_pool(name="av_psum", bufs=2, space="PSUM") as av_psum, \
                 tc.tile_pool(name="exp_pool", bufs=5) as exp_pool:

                # re-warm the PE right as the AllGather lands: a contiguous
                # dummy-matmul burst gated on the stats DMA flips the HAM
                # clock gate to 8/8 before the qkv matmuls start
                warm2 = mm_psum.tile([P, 512], F32, name="ps_mm")
                for _ in range(70):
                    nc.tensor.matmul(warm2[0:1, 0:16], lhsT=junk[:, 0:1],
                                     rhs=stats_g[:, 0].rearrange("p r s -> p (r s)"),
                                     start=True, stop=True)

                # effective biases: b' = b - W_scaled @ mu  (q also * SCALE).
                # k-bias is dropped: it adds a per-query constant to every
                # logit, which softmax shift-invariance cancels exactly.
                pq = mm_psum.tile([P, 512], F32, name="ps_mm")
                for cc in range(CCH):
                    nc.tensor.matmul(pq[:, 0:1], lhsT=wq_s[:, cc],
                                     rhs=mu_bf[:, cc : cc + 1],
                                     start=(cc == 0), stop=(cc == CCH - 1))
                pv = mm_psum.tile([P, 512], F32, name="ps_mm")
                for cc in range(CCH):
                    nc.tensor.matmul(pv[0:HD, 0:1], lhsT=wv_s[:, cc],
                                     rhs=mu_bf[:, cc : cc + 1],
                                     start=(cc == 0), stop=(cc == CCH - 1))
                nc.vector.tensor_tensor(bq_eff, bq_sb, pq[:, 0:1], ALU.subtract)
                nc.vector.tensor_scalar_mul(bq_eff, bq_eff, SCALE)
                nc.vector.tensor_tensor(bv_eff, bv_sb, pv[0:HD, 0:1], ALU.subtract)
                for i in range(4):
                    nc.vector.tensor_copy(bv4[32 * i : 32 * i + 32], bv_eff)

                av_tiles = {}
                ex_tiles = {}
                state = {"junk_ps": None}

                def fill_pe(n):
                    # tiny matmuls that keep the PE activity monitor fed while
                    # the exp engines catch up; they land in a dead mm_psum
                    # tile and are never read
                    jp = state["junk_ps"]
                    if jp is None:
                        return
                    for _ in range(n):
                        nc.tensor.matmul(
                            jp[0:1, 0:FILL_W], lhsT=junk[:, 0:1],
                            rhs=junk[:, 0:FILL_W], start=True, stop=True,
                            skip_group_check=True,
                        )

                def _exp(i, sc):
                    ex = exp_pool.tile([P, 1024], BF16, name="ex")
                    if exp_on_act[i]:
                        nc.scalar.activation(ex, sc[:], AF.Exp)
                    else:
                        # Schraudolph bf16 exp on DVE: one fused mul-add with
                        # int16 convert-on-write, bit-viewed as bf16
                        nc.vector.tensor_scalar(
                            ex.bitcast(I16)[:], sc[:], SCH_A, SCH_B,
                            op0=ALU.mult, op1=ALU.add,
                        )
                    ex_tiles[i] = ex

                def attn_score(i):
                    un = units[i]
                    if un[0] == "old":
                        # block-0 flavor: 2 chunks x 512 queries (2-way row
                        # packing), interleaves cleanly with the qkv ramp
                        b, s = un[1], un[2]
                        if b > 0:
                            fill_pe(FILL_SC)
                        if s == 0:
                            av_tiles[b] = av_psum.tile([97, 512], F32, name="av")
                        sc = sc_psum.tile([P, 1024], F32, name="sc")
                        for r in range(2):
                            cm = 2 * s + r
                            nc.tensor.matmul(
                                sc[:, r * 512 : (r + 1) * 512],
                                lhsT=kT[32 * r : 32 * (r + 1), cm * P : (cm + 1) * P],
                                rhs=qT[32 * r : 32 * (r + 1), b * 512 : (b + 1) * 512],
                                start=True, stop=True,
                                tile_position=(32 * r, 0),
                            )
                        _exp(i, sc)
                        return
                    # steady-state flavor: 4 chunks x 256-query strip, 4-way
                    # row-group packing (concurrent PE sub-arrays)
                    b, h, u = un[1], un[2], un[3]
                    fill_pe(FILL_SC)
                    if u == 0 and h == 0:
                        av_tiles[b] = av_psum.tile([97, 512], F32, name="av")
                    sc = sc_psum.tile([P, 1024], F32, name="sc")
                    q0 = b * 512 + h * 256
                    for r in range(4):
                        cm = 4 * u + r
                        rp = SC_POS[r]
                        nc.tensor.matmul(
                            sc[:, r * 256 : (r + 1) * 256],
                            lhsT=kT[rp : rp + 32, cm * P : (cm + 1) * P],
                            rhs=qT[rp : rp + 32, q0 : q0 + 256],
                            start=True, stop=True,
                            tile_position=(rp, 0),
                        )
                    _exp(i, sc)

                def attn_av(i):
                    un = units[i]
                    ex = ex_tiles.pop(i)
                    if un[0] == "old":
                        b, s = un[1], un[2]
                        if b > 0:
                            fill_pe(FILL_AV)
                        av = av_tiles[b]
                        for r in range(2):
                            cm = 2 * s + r
                            nc.tensor.matmul(
                                av[64 * r : 64 * r + HD + 1, :],
                                lhsT=v_sb[:, cm * (HD + 1) : (cm + 1) * (HD + 1)],
                                rhs=ex[:, r * 512 : (r + 1) * 512],
                                start=(s == 0),
                                stop=(s == 15),
                                tile_position=(0, 64 * r),
                                skip_group_check=True,
                            )
                        return
                    b, h, u = un[1], un[2], un[3]
                    fill_pe(FILL_AV)
                    av = av_tiles[b]
                    for r in range(4):
                        cm = 4 * u + r
                        cp = 64 * (r % 2)
                        nc.tensor.matmul(
                            av[cp : cp + HD + 1, h * 256 : (h + 1) * 256],
                            lhsT=v_sb[:, cm * (HD + 1) : (cm + 1) * (HD + 1)],
                            rhs=ex[:, r * 256 : (r + 1) * 256],
                            start=(u == 0 and r < 2),
                            stop=(u == 7 and r >= 2),
                            tile_position=(0, cp),
                            skip_group_check=True,
                        )

                epilogue_q = []

                def attn_block_end(b):
                    # queue the evacuate/combine/normalize pieces; one piece
                    # is emitted per subsequent unit so the DVE queue never
                    # bulges and stalls the exp pipeline
                    av = av_tiles.pop(b)
                    q0 = b % 4
                    g = b // 4
                    # DVE TensorTensor may read only one PSUM operand, so ACT
                    # first evacuates the second col-packed half to SBUF
                    tmp33 = work.tile([HD + 1, 512], F32, name="tmp33")
                    epilogue_q.append(lambda: nc.scalar.activation(
                        tmp33, av[64 : 64 + HD + 1, :], AF.Identity))
                    epilogue_q.append(lambda: nc.vector.tensor_tensor(
                        numer4[32 * q0 : 32 * q0 + HD, g, :],
                        av[0:HD, :], tmp33[0:HD, :], ALU.add))
                    epilogue_q.append(lambda: nc.vector.tensor_tensor(
                        den4[32 * q0 : 32 * q0 + 1, g, :],
                        av[HD : HD + 1, :], tmp33[HD : HD + 1, :], ALU.add))
                    if b % 4 == 3:
                        # normalize this group of 4 blocks; quake reciprocal
                        # seed + 1 Newton step stays DVE-only and cheap
                        dg = den4[:, g, :]
                        rg = recip4[:, g, :]
                        bg = bcast4[:, g, :]
                        tg = t512[:, g, :]
                        if USE_RECIP_FAST:
                            epilogue_q.append(
                                lambda: nc.vector.reciprocal_approx_fast(rg, dg))
                        else:
                            epilogue_q.append(lambda: nc.vector.tensor_scalar(
                                rg.bitcast(I32), dg.bitcast(I32), 0, -1,
                                op0=ALU.arith_shift_right, op1=ALU.bitwise_xor))
                            epilogue_q.append(lambda: nc.vector.tensor_scalar_add(
                                rg.bitcast(I32), rg.bitcast(I32), RECIP_MAGIC + 1))
                            epilogue_q.append(lambda: nc.vector.tensor_tensor(
                                tg, dg, rg, ALU.mult))
                            epilogue_q.append(lambda: nc.vector.tensor_scalar(
                                tg, tg, -1.0, 2.0, op0=ALU.mult, op1=ALU.add))
                            epilogue_q.append(lambda: nc.vector.tensor_tensor(
                                rg, rg, tg, ALU.mult))
                        epilogue_q.append(lambda: nc.vector.stream_shuffle(
                            bg, rg, mask=[0] * 32))
                        # group 0's final normalize runs on the (otherwise
                        # idle) Pool engine, far from the critical tail
                        eng = nc.gpsimd if (g == 0 and USE_POOL_NORM) else nc.vector
                        epilogue_q.append(lambda eng=eng: eng.tensor_tensor(
                            out4[:, g, :], numer4[:, g, :], bg, ALU.mult))
                        epilogue_q.append(lambda eng=eng: eng.tensor_scalar_add(
                            out4[:, g, :], out4[:, g, :], bv4))
                        for j in range(4 * g, 4 * g + 4):
                            epilogue_q.append(
                                lambda j=j: (nc.sync if j % 2 == 0 else nc.scalar)
                                .dma_start(
                                    a2a_in[j],
                                    out4[32 * (j % 4) : 32 * (j % 4) + HD, j // 4, :],
                                ))

                def qkv_rank(r):
                    for cc in range(CCH):
                        nc.sync.dma_start(
                            y_full[:, cc, r * NS : (r + 1) * NS],
                            ago[:, cc, r, 0:NS],
                        )
                    # v for the rank's 4 m-chunks
                    for cl in range(4):
                        mb = 4 * r + cl
                        psv = mm_psum.tile([P, 512], F32, name="ps_mm")
                        for cc in range(CCH):
                            nc.tensor.matmul(
                                psv[:, 0:HD],
                                lhsT=y_full[:, cc, mb * P : (mb + 1) * P],
                                rhs=wv_s[:, cc],
                                start=(cc == 0), stop=(cc == CCH - 1),
                            )
                        if cl % 2 == 0:
                            nc.vector.tensor_copy(
                                v_sb[:, mb * (HD + 1) : mb * (HD + 1) + HD],
                                psv[:, 0:HD],
                            )
                        else:
                            nc.scalar.activation(
                                v_sb[:, mb * (HD + 1) : mb * (HD + 1) + HD],
                                psv[:, 0:HD], AF.Identity,
                            )
                    psq = mm_psum.tile([P, 512], F32, name="ps_mm")
                    for cc in range(CCH):
                        nc.tensor.matmul(
                            psq, lhsT=wq_s[:, cc],
                            rhs=y_full[:, cc, r * 512 : (r + 1) * 512],
                            start=(cc == 0), stop=(cc == CCH - 1),
                        )
                    nc.scalar.activation(
                        qT[:, r * 512 : (r + 1) * 512], psq, AF.Identity,
                        bias=bq_eff, scale=SCALE,
                    )
                    psk = mm_psum.tile([P, 512], F32, name="ps_mm")
                    for cc in range(CCH):
                        nc.tensor.matmul(
                            psk, lhsT=wk_s[:, cc],
                            rhs=y_full[:, cc, r * 512 : (r + 1) * 512],
                            start=(cc == 0), stop=(cc == CCH - 1),
                        )
                    nc.vector.tensor_copy(kT[:, r * 512 : (r + 1) * 512], psk)

                # unit stream: block 0 uses the baseline (b, s) shape so the
                # qkv ramp pairing holds (unit s needs rank s//2's kT/qT);
                # blocks 1-7 use (b, h, u) with h-outer so each 256-col AV
                # strip's psum accumulation completes before the next strip
                # starts (CoreSim's pending-zero regions are 2KB-granular).
                # scores run two units ahead of AVs so both exp engines stay
                # saturated; filler matmuls pad the PE's exp-wait gaps.
                if ALL_OLD:
                    units = [("old", b, s) for b in range(NNB)
                             for s in range(16)]
                else:
                    units = [("old", 0, s) for s in range(16)] + [
                        ("new", b, h, u) for b in range(1, NNB)
                        for h in range(2) for u in range(8)
                    ]
                exp_on_act = _exp_schedule(units)

                def prework(i):
                    un = units[i]
                    if un[0] == "old" and un[1] == 0 and un[2] % 2 == 0:
                        qkv_rank(un[2] // 2)
                    if un[1] == 1 and un[2] == 0 and (un[0] == "old" or un[3] == 0):
                        # qkv ramp done: park a dead mm_psum tile for fillers
                        state["junk_ps"] = mm_psum.tile([P, 512], F32, name="ps_mm")

                def is_block_last(un):
                    return (un[2] == 15) if un[0] == "old" else (
                        un[2] == 1 and un[3] == 7)

                for i in range(2):
                    prework(i)
                    attn_score(i)
                for i, un in enumerate(units):
                    if i + 2 < len(units):
                        prework(i + 2)
                        attn_score(i + 2)
                    attn_av(i)
                    if is_block_last(un):
                        attn_block_end(un[1])
                    if epilogue_q:
                        epilogue_q.pop(0)()
                while epilogue_q:
                    epilogue_q.pop(0)()

            # ---- phase 5: all-to-all + projection ----
            nc.gpsimd.collective_compute(
                "AllToAll",
                ALU.bypass,
                replica_groups=[list(range(NCORES))],
                ins=[a2a_in[:].opt()],
                outs=[a2a_out[:].opt()],
            )
            cat = a2a_out[:].rearrange("h d f -> (h d) f")  # [256, 512]
            c_sb = persist.tile([P, CCH, NS], BF16, name="c_sb")
            for cc in range(CCH):
                (nc.sync if cc == 0 else nc.scalar).dma_start(
                    c_sb[:, cc], cat[cc * P : (cc + 1) * P]
                )
            out_sb = persist.tile([P, CCH, NS], F32, name="out_sb")
            with tc.tile_pool(name="proj_psum", bufs=2, space="PSUM") as proj_psum:
                for ob in range(CCH):
                    psp = proj_psum.tile([P, NS], F32, name="ps_proj")
                    for cc in range(CCH):
                        nc.tensor.matmul(
                            psp,
                            lhsT=wproj_sb[:, cc, ob * P : (ob + 1) * P],
                            rhs=c_sb[:, cc],
                            start=(cc == 0), stop=(cc == CCH - 1),
                        )
                    nc.scalar.activation(
                        out_sb[:, ob], psp, AF.Identity,
                        bias=bproj_sb[:, ob : ob + 1], scale=1.0,
                    )
                    (nc.sync if ob == 0 else nc.scalar).dma_start(
                        out_d[ob], out_sb[:, ob])

    nc.compile()
    return nc


def _host_prep(x, w_dw, b_dw, w_qkv, b_qkv, w_proj, b_proj):
    """Build per-core in_maps from full inputs."""
    x = np.asarray(x, np.float32)
    w_dw = np.asarray(w_dw, np.float32)
    b_dw = np.asarray(b_dw, np.float32)
    w_qkv = np.asarray(w_qkv, np.float32)
    b_qkv = np.asarray(b_qkv, np.float32)
    w_proj = np.asarray(w_proj, np.float32)
    b_proj = np.asarray(b_proj, np.float32)

    xs = x[0]  # [C, 16, 16, 16]
    # diag conv weights: [CCH, 27, P, P]
    dw_diag = np.zeros((CCH, 27, P, P), np.float32)
    for cc in range(CCH):
        for t, (dh, dw_, dd) in enumerate(TAPS):
            np.fill_diagonal(dw_diag[cc, t], w_dw[cc * P : (cc + 1) * P, 0, dh, dw_, dd])
    dw_diag = dw_diag.astype(bf16)
    b_dw_s = b_dw.reshape(CCH, P, 1)

    wproj_t = np.ascontiguousarray(w_proj.T).reshape(CCH, P, C).astype(bf16)
    bproj_s = b_proj.reshape(CCH, P, 1)

    in_maps = []
    for h in range(NCORES):
        # padded x slab: global h rows 2h-1 .. 2h+2, padded w/d
        xp = np.zeros((C, 4, 18, 18), np.float32)
        for hl in range(4):
            hg = 2 * h - 1 + hl
            if 0 <= hg < HWD:
                xp[:, hl, 1:17, 1:17] = xs[:, hg]
        xp = xp.reshape(CCH, P, 4 * 18 * 18).astype(bf16)

        wq_h = w_qkv[h * HD : (h + 1) * HD]  # [32, 256]
        wk_h = w_qkv[C + h * HD : C + (h + 1) * HD]
        wv_h = w_qkv[2 * C + h * HD : 2 * C + (h + 1) * HD]
        wq_rep = np.tile(wq_h.T, (1, 4)).reshape(C, P)  # [256, 128]
        wk_rep = np.tile(wk_h.T, (1, 4)).reshape(C, P)
        in_maps.append({
            "x_pad": xp,
            "dw_diag": dw_diag,
            "b_dw": b_dw_s,
            "wq": wq_rep.reshape(CCH, P, P).astype(bf16),
            "wk": wk_rep.reshape(CCH, P, P).astype(bf16),
            "wv": np.ascontiguousarray(wv_h.T).reshape(CCH, P, HD).astype(bf16),
            "bq": np.tile(b_qkv[h * HD : (h + 1) * HD], 4).reshape(P, 1).astype(np.float32),
            "bv": b_qkv[2 * C + h * HD : 2 * C + (h + 1) * HD].reshape(HD, 1).astype(np.float32),
            "wproj": wproj_t,
            "bproj": bproj_s,
        })
    return in_maps


def kernel(**inputs):
    if "nc" not in _cache:
        _cache["nc"] = _build_graph()
    nc = _cache["nc"]
    in_maps = _host_prep(**inputs)
    res = bass_utils.run_bass_kernel_spmd(nc, in_maps, core_ids=list(range(NCORES)))
    slices = [res.results[j]["out"].reshape(C, NS) for j in range(NCORES)]
    full = np.concatenate(slices, axis=1)  # [256, 4096]
    return full.reshape(1, C, HWD, HWD, HWD).astype(np.float32)


if __name__ == "__main__":
    nc = _build_graph()
    print("graph built + compiled OK")


# revision 19
# speedup vs baseline: 1.3481x; 1.1101x over previous
"""Distributed Bass kernel for nn_Attention_80908593922315 on 8 TRN2 NeuronCores.

Sharding: head-parallel attention (core h owns head h) + spatial-parallel
conv/proj (core j owns flattened-spatial slice [512j, 512j+512)).

Pipeline per core:
  0. PE warm-up dummies (HAM un-throttle) while inputs stream in
  1. depthwise 3x3x3 conv on PE (27 accumulating diag-matmuls over a
     zero-padded local slab), + per-channel partial stats via ACT accum_out
  2. AllGather of (y, stats) -> full unnormalized y + global instance-norm
     stats; normalization is folded into the QKV weights/biases
  3. per-rank pipeline: y slab DMA -> v/q/k matmuls interleaved with block-0
     attention units; dummy burst gated on the stats DMA re-warms the PE
  4. attention: per unit (4 key-chunks x 256-query strip) scores via FOUR
     K=32 row-group-packed matmuls (concurrent on PE sub-arrays) -> exp
     split ACT (table exp) / DVE (Schraudolph int16 bit-trick), statically
     load-balanced -> AV accumulation [97, 256-strip] x2 col-packed halves
     (row 32 = sum of exp = denominator); small filler matmuls pad PE gaps
     so the HAM clock gate stays at 8/8
  5. per-group normalize (single-op DVE reciprocal + partition broadcast),
     AllToAll (head-major -> spatial slices), 1x1 proj on the local 512
     columns. k-bias is dropped entirely (softmax shift-invariance).
"""

import sys

if "/opt/trn_rl_repo" not in sys.path:
    sys.path.insert(0, "/opt/trn_rl_repo")

import numpy as np
import ml_dtypes

import concourse.bass as bass
import concourse.bacc as bacc
import concourse.tile as tile
import concourse.mybir as mybir
from concourse import bass_utils

BF16 = mybir.dt.bfloat16
F32 = mybir.dt.float32
I16 = mybir.dt.int16
I32 = mybir.dt.int32
AF = mybir.ActivationFunctionType
ALU = mybir.AluOpType
bf16 = ml_dtypes.bfloat16

NCORES = 8
C = 256
NH = 8
HD = 32
HWD = 16
N = HWD * HWD * HWD  # 4096
NS = N // NCORES  # 512 spatial per core
P = 128
CCH = C // P  # 2 channel chunks
EPS = 1e-5
SCALE = HD ** -0.5
NMB = N // P  # 32 m-chunks
NNB = N // 512  # 8 n-blocks
TAPS = [(a, b, c) for a in range(3) for b in range(3) for c in range(3)]

# Schraudolph bf16 exp: exp(x) ~= bitcast_bf16(int16(A*x + B))
SCH_A = 128.0 / float(np.log(2.0))
SCH_B = 16255.5

# PE filler matmuls per unit (before scores / before AV) to keep the HAM
# clock gate at K=8/8 during exp-bound stretches; [1, FILL_W] each
import os
FILL_SC = int(os.environ.get("K_FILL_SC", "0"))
FILL_AV = int(os.environ.get("K_FILL_AV", "0"))
FILL_W = 96
USE_POOL_NORM = os.environ.get("K_POOL_NORM", "0") == "1"
USE_RECIP_FAST = os.environ.get("K_RECIP_FAST", "0") == "1"
# quake reciprocal seed: r0 = bitcast(MAGIC - i) = bitcast((i ^ -1) + MAGIC+1)
RECIP_MAGIC = 0x7EF311C3
ALL_OLD = os.environ.get("K_ALL_OLD", "1") == "1"
# row-group PE positions for the 4 score matmuls of a new-flavor unit
SC_POS = [int(x) for x in os.environ.get("K_SC_POS", "0,32,64,96").split(",")]

_cache = {}


def _exp_schedule(units):
    """Static greedy ACT/DVE assignment per exp tile (2 slots per unit).

    Models per-tile engine-busy in us: ACT ~0.996, DVE ~1.128, plus the
    side work each engine owes (qkv ramp writes, epilogue).
    """
    A = 0.0
    D = 0.0
    sched = []

    def pick():
        nonlocal A, D
        if A + 0.996 <= D + 1.128:
            sched.append(True)
            A += 0.996
        else:
            sched.append(False)
            D += 1.128

    for un in units:
        if un[0] == "old" and un[1] == 0 and un[2] % 2 == 0:
            A += 0.85  # qT write + v-half
            D += 0.65  # kT write + v-half
        if un[0] == "old":
            pick()
            sched.append(False)  # unused slot
        else:
            pick()
            pick()
        b = un[1]
        is_last = (un[2] == 15) if un[0] == "old" else (un[2] == 7)
        if is_last:
            A += 0.6  # block-end tmp33 evacuate
            D += 1.32  # block-end combine adds
            if b % 4 == 3:
                D += 1.3  # recip + shuffle
                if b // 4 == 1:
                    D += 1.2  # group-1 out4 normalize stays on DVE
    return sched


def _build_graph():
    nc = bacc.Bacc(
        "TRN2",
        target_bir_lowering=False,
        debug=False,
        enable_asserts=True,
        num_devices=NCORES,
    )

    # ---- I/O declarations (per-core shards) ----
    x_pad = nc.dram_tensor("x_pad", [CCH, P, 4 * 18 * 18], BF16, kind="ExternalInput").ap()
    dw_diag = nc.dram_tensor("dw_diag", [CCH, 27, P, P], BF16, kind="ExternalInput").ap()
    b_dw = nc.dram_tensor("b_dw", [CCH, P, 1], F32, kind="ExternalInput").ap()
    wq_d = nc.dram_tensor("wq", [CCH, P, P], BF16, kind="ExternalInput").ap()
    wk_d = nc.dram_tensor("wk", [CCH, P, P], BF16, kind="ExternalInput").ap()
    wv_d = nc.dram_tensor("wv", [CCH, P, HD], BF16, kind="ExternalInput").ap()
    bq_d = nc.dram_tensor("bq", [P, 1], F32, kind="ExternalInput").ap()
    bv_d = nc.dram_tensor("bv", [HD, 1], F32, kind="ExternalInput").ap()
    wproj_d = nc.dram_tensor("wproj", [CCH, P, C], BF16, kind="ExternalInput").ap()
    bproj_d = nc.dram_tensor("bproj", [CCH, P, 1], F32, kind="ExternalInput").ap()
    out_d = nc.dram_tensor("out", [CCH, P, NS], F32, kind="ExternalOutput").ap()

    with tile.TileContext(nc) as tc:
        with tc.tile_pool(name="persist", bufs=1) as persist, \
             tc.tile_pool(name="dram", bufs=1, space="DRAM") as dram, \
             tc.tile_pool(name="work", bufs=4) as work:

            # ---- PE warm-up: keep HAM at K=8/8 from the start so the conv
            # matmuls run at 2.4 GHz; covers the input-DMA window.
            junk = persist.tile([P, P], BF16, name="junk")
            nc.vector.memset(junk[:], 0.0)

            # ---- input DMAs, conv-critical first, spread across queues ----
            xp_sb = persist.tile([P, CCH, 4 * 18 * 18], BF16, name="xp_sb")
            dwd_sb = persist.tile([P, CCH, 27, P], BF16, name="dwd_sb")
            # sync queue: chunk-0 conv inputs (first matmul gate)
            nc.sync.dma_start(xp_sb[:, 0], x_pad[0])
            for g in range(4):
                tg = slice(7 * g, min(27, 7 * g + 7))
                nc.sync.dma_start(
                    dwd_sb[:, 0, tg], dw_diag[0, tg].rearrange("t p q -> p t q")
                )
            # scalar queue: chunk-1 conv inputs (ACT is idle this early)
            nc.scalar.dma_start(xp_sb[:, 1], x_pad[1])
            for g in range(4):
                tg = slice(7 * g, min(27, 7 * g + 7))
                nc.scalar.dma_start(
                    dwd_sb[:, 1, tg], dw_diag[1, tg].rearrange("t p q -> p t q")
                )
            bdw_sb = persist.tile([P, CCH], F32, name="bdw_sb")
            for cc in range(CCH):
                nc.scalar.dma_start(bdw_sb[:, cc : cc + 1], b_dw[cc])
            # gpsimd queue: qkv weights (needed right after the AllGather)
            wq_sb = persist.tile([P, CCH, P], BF16, name="wq_sb")
            wk_sb = persist.tile([P, CCH, P], BF16, name="wk_sb")
            wv_sb = persist.tile([P, CCH, HD], BF16, name="wv_sb")
            for cc in range(CCH):
                nc.gpsimd.dma_start(wq_sb[:, cc], wq_d[cc])
                nc.gpsimd.dma_start(wk_sb[:, cc], wk_d[cc])
                nc.gpsimd.dma_start(wv_sb[:, cc], wv_d[cc])
            bq_sb = persist.tile([P, 1], F32, name="bq_sb")
            bv_sb = persist.tile([HD, 1], F32, name="bv_sb")
            nc.gpsimd.dma_start(bq_sb[:], bq_d)
            nc.gpsimd.dma_start(bv_sb[:], bv_d)
            wproj_sb = persist.tile([P, CCH, C], BF16, name="wproj_sb")
            bproj_sb = persist.tile([P, CCH], F32, name="bproj_sb")
            for cc in range(CCH):
                nc.gpsimd.dma_start(wproj_sb[:, cc], wproj_d[cc])
                nc.gpsimd.dma_start(bproj_sb[:, cc : cc + 1], bproj_d[cc])

            # preload the exp activation table early (ACT idle anyway)
            exp_dummy = work.tile([1, 16], BF16, name="exp_dummy")
            nc.vector.memset(exp_dummy[:], 0.0)
            nc.scalar.activation(exp_dummy, exp_dummy, AF.Exp)

            # v ones-columns (softmax denominator rows) set up early
            v_sb = persist.tile([P, NMB * (HD + 1)], BF16, name="v_sb")
            nc.vector.memset(v_sb[:], 1.0)

            # ---- phase 1+2: depthwise conv, then one AllGather of (y, stats)
            y_sb = persist.tile([P, CCH, 514], BF16, name="y_sb")
            stats_sb = persist.tile([P, CCH, 2], F32, name="stats_sb")
            sq_junk = work.tile([P, NS], BF16, name="sq_junk")
            ag_in = dram.tile([P, CCH * 514], BF16, name="ag_in")
            ag_out = dram.tile([NCORES, P, CCH * 514], BF16, name="ag_out",
                               addr_space="Shared")
            with tc.tile_pool(name="warm_psum", bufs=1, space="PSUM") as warm_psum, \
                 tc.tile_pool(name="conv_psum", bufs=2, space="PSUM") as conv_psum:
                wps = warm_psum.tile([1, P], F32, name="wps")
                for _ in range(100):
                    nc.tensor.matmul(wps, lhsT=junk[:, 0:1], rhs=junk[:],
                                     start=True, stop=True)
                for cc in range(CCH):
                    ps = conv_psum.tile([P, NS], F32, name="ps_conv")
                    x4 = xp_sb[:, cc].rearrange("p (a b c) -> p a b c", b=18, c=18)
                    for t, (dh, dw_, dd) in enumerate(TAPS):
                        nc.tensor.matmul(
                            ps,
                            lhsT=dwd_sb[:, cc, t],
                            rhs=x4[:, dh : dh + 2, dw_ : dw_ + 16, dd : dd + 16],
                            start=(t == 0),
                            stop=(t == 26),
                        )
                    nc.scalar.activation(
                        y_sb[:, cc, 0:NS], ps, AF.Identity,
                        bias=bdw_sb[:, cc : cc + 1], scale=1.0,
                        accum_out=stats_sb[:, cc, 0:1],
                    )
                    nc.scalar.activation(
                        sq_junk, ps, AF.Square,
                        bias=bdw_sb[:, cc : cc + 1], scale=1.0,
                        accum_out=stats_sb[:, cc, 1:2],
                    )
                    nc.vector.tensor_copy(y_sb[:, cc, NS : NS + 2], stats_sb[:, cc])
                    # per-chunk bounce, y first (its transfer hides under the
                    # stats chain), then the 2 stats columns gate the trigger
                    q = nc.sync if cc == 0 else nc.scalar
                    q.dma_start(
                        ag_in[:, cc * 514 : cc * 514 + NS], y_sb[:, cc, 0:NS]
                    )
                    q.dma_start(
                        ag_in[:, cc * 514 + NS : (cc + 1) * 514],
                        y_sb[:, cc, NS : NS + 2],
                    )
            nc.gpsimd.collective_compute(
                "AllGather",
                ALU.bypass,
                replica_groups=[list(range(NCORES))],
                ins=[ag_in[:].opt()],
                outs=[ag_out[:].opt()],
            )

            y_full = persist.tile([P, CCH, N], BF16, name="y_full")
            stats_g = work.tile([P, CCH, NCORES, 2], BF16, name="stats_g")
            ssum = work.tile([P, CCH, 2], F32, name="ssum")
            ago = ag_out[:].rearrange("r p (q f) -> p q r f", q=CCH)  # [128,2,8,514]
            # stats first (tiny) so the weight-fold chain runs during the
            # 2MB y_full loads instead of queueing behind them
            for cc in range(CCH):
                nc.sync.dma_start(stats_g[:, cc], ago[:, cc, :, NS : NS + 2])
            for cc in range(CCH):
                nc.vector.reduce_sum(
                    ssum[:, cc],
                    stats_g[:, cc].rearrange("p r s -> p s r"),
                    axis=mybir.AxisListType.X,
                )

            mu = persist.tile([P, CCH], F32, name="mu")
            rstd = persist.tile([P, CCH], F32, name="rstd")
            t_a = work.tile([P, 1], F32, name="t_a")
            t_b = work.tile([P, 1], F32, name="t_b")
            t_c = work.tile([P, 1], F32, name="t_c")
            for cc in range(CCH):
                mcc = mu[:, cc : cc + 1]
                rcc = rstd[:, cc : cc + 1]
                nc.vector.tensor_scalar_mul(mcc, ssum[:, cc, 0:1], 1.0 / N)
                nc.vector.tensor_scalar_mul(t_a, ssum[:, cc, 1:2], 1.0 / N)  # E[y^2]
                nc.vector.tensor_tensor(t_b, mcc, mcc, ALU.mult)  # mu^2
                nc.vector.tensor_tensor(t_a, t_a, t_b, ALU.subtract)  # var
                nc.vector.tensor_scalar_add(t_a, t_a, EPS)  # var+eps
                # quake rsqrt seed (no ACT table set needed) + 2 Newton steps
                rci = rcc.bitcast(I32)
                nc.vector.tensor_scalar(
                    rci, t_a.bitcast(I32), 1, -1,
                    op0=ALU.arith_shift_right, op1=ALU.bitwise_xor,
                )
                nc.vector.tensor_scalar_add(rci, rci, 0x5F3759DF + 1)
                for _ in range(2):
                    nc.vector.tensor_tensor(t_b, rcc, rcc, ALU.mult)
                    nc.vector.tensor_tensor(t_c, t_a, t_b, ALU.mult)
                    nc.vector.tensor_scalar(
                        t_c, t_c, -0.5, 1.5, op0=ALU.mult, op1=ALU.add
                    )
                    nc.vector.tensor_tensor(rcc, rcc, t_c, ALU.mult)

            # fold norm scale into qkv weights
            wq_s = persist.tile([P, CCH, P], BF16, name="wq_s")
            wk_s = persist.tile([P, CCH, P], BF16, name="wk_s")
            wv_s = persist.tile([P, CCH, HD], BF16, name="wv_s")
            mu_bf = work.tile([P, CCH], BF16, name="mu_bf")
            nc.vector.tensor_copy(mu_bf[:], mu[:])
            for cc in range(CCH):
                nc.vector.tensor_scalar_mul(wv_s[:, cc], wv_sb[:, cc], rstd[:, cc : cc + 1])
                nc.vector.tensor_scalar_mul(wq_s[:, cc], wq_sb[:, cc], rstd[:, cc : cc + 1])
                nc.vector.tensor_scalar_mul(wk_s[:, cc], wk_sb[:, cc], rstd[:, cc : cc + 1])

            # ---- phases 3+4 interleaved: per-rank qkv + attention ----
            qT = persist.tile([P, N], BF16, name="qT")
            kT = persist.tile([P, N], BF16, name="kT")
            numer4 = persist.tile([P, 2, 512], F32, name="numer4")
            den4 = persist.tile([P, 2, 512], F32, name="den4")
            recip4 = persist.tile([P, 2, 512], F32, name="recip4")
            bcast4 = persist.tile([P, 2, 512], F32, name="bcast4")
            t512 = persist.tile([P, 2, 512], F32, name="t512")
            nc.vector.memset(den4[:], 1.0)
            out4 = persist.tile([P, 2, 512], BF16, name="out4")
            a2a_in = dram.tile([NCORES, HD, NS], BF16, name="a2a_in")
            a2a_out = dram.tile([NCORES, HD, NS], BF16, name="a2a_out")
            bq_eff = persist.tile([P, 1], F32, name="bq_eff")
            bv_eff = persist.tile([HD, 1], F32, name="bv_eff")
            bv4 = persist.tile([P, 1], F32, name="bv4")

            # PSUM layout (8 banks): mm_psum 2x[128,512] + sc_psum 2x[128,1024]
            # + av_psum 2x[97,512] coexist so attention overlaps the qkv ramp.
            with tc.tile_pool(name="mm_psum", bufs=2, space="PSUM") as mm_psum, \
                 tc.tile_pool(name="sc_psum", bufs=2, space="PSUM") as sc_psum, \
                 tc.tile_pool(name="av_psum", bufs=2, space="PSUM") as av_psum, \
                 tc.tile_pool(name="exp_pool", bufs=5) as exp_pool:

                # re-warm the PE right as the AllGather lands: a contiguous
                # dummy-matmul burst gated on the stats DMA flips the HAM
                # clock gate to 8/8 before the qkv matmuls start
                warm2 = mm_psum.tile([P, 512], F32, name="ps_mm")
                for _ in range(70):
                    nc.tensor.matmul(warm2[0:1, 0:16], lhsT=junk[:, 0:1],
                                     rhs=stats_g[:, 0].rearrange("p r s -> p (r s)"),
                                     start=True, stop=True)

                # effective biases: b' = b - W_scaled @ mu  (q also * SCALE).
                # k-bias is dropped: it adds a per-query constant to every
                # logit, which softmax shift-invariance cancels exactly.
                pq = mm_psum.tile([P, 512], F32, name="ps_mm")
                for cc in range(CCH):
                    nc.tensor.matmul(pq[:, 0:1], lhsT=wq_s[:, cc],
                                     rhs=mu_bf[:, cc : cc + 1],
                                     start=(cc == 0), stop=(cc == CCH - 1))
                pv = mm_psum.tile([P, 512], F32, name="ps_mm")
                for cc in range(CCH):
                    nc.tensor.matmul(pv[0:HD, 0:1], lhsT=wv_s[:, cc],
                                     rhs=mu_bf[:, cc : cc + 1],
                                     start=(cc == 0), stop=(cc == CCH - 1))
                nc.vector.tensor_tensor(bq_eff, bq_sb, pq[:, 0:1], ALU.subtract)
                nc.vector.tensor_scalar_mul(bq_eff, bq_eff, SCALE)
                nc.vector.tensor_tensor(bv_eff, bv_sb, pv[0:HD, 0:1], ALU.subtract)
                for i in range(4):
                    nc.vector.tensor_copy(bv4[32 * i : 32 * i + 32], bv_eff)

                av_tiles = {}
                ex_tiles = {}
                state = {"junk_ps": None}

                def fill_pe(n):
                    # tiny matmuls that keep the PE activity monitor fed while
                    # the exp engines catch up; they land in a dead mm_psum
                    # tile and are never read
                    jp = state["junk_ps"]
                    if jp is None:
                        return
                    for _ in range(n):
                        nc.tensor.matmul(
                            jp[0:1, 0:FILL_W], lhsT=junk[:, 0:1],
                            rhs=junk[:, 0:FILL_W], start=True, stop=True,
                            skip_group_check=True,
                        )

                def _exp(i, sc):
                    ex = exp_pool.tile([P, 1024], BF16, name="ex")
                    if exp_on_act[2 * i]:
                        nc.scalar.activation(ex, sc[:], AF.Exp)
                    else:
                        # Schraudolph bf16 exp on DVE: one fused mul-add with
                        # int16 convert-on-write, bit-viewed as bf16
                        nc.vector.tensor_scalar(
                            ex.bitcast(I16)[:], sc[:], SCH_A, SCH_B,
                            op0=ALU.mult, op1=ALU.add,
                        )
                    ex_tiles[i] = ex

                def attn_score(i):
                    un = units[i]
                    if un[0] == "old":
                        # block-0 flavor: 2 chunks x 512 queries (2-way row
                        # packing), interleaves cleanly with the qkv ramp
                        b, s = un[1], un[2]
                        if b > 0:
                            fill_pe(FILL_SC)
                        if s == 0:
                            av_tiles[b] = av_psum.tile([97, 512], F32, name="av")
                        sc = sc_psum.tile([P, 1024], F32, name="sc")
                        for r in range(2):
                            cm = 2 * s + r
                            nc.tensor.matmul(
                                sc[:, r * 512 : (r + 1) * 512],
                                lhsT=kT[32 * r : 32 * (r + 1), cm * P : (cm + 1) * P],
                                rhs=qT[32 * r : 32 * (r + 1), b * 512 : (b + 1) * 512],
                                start=True, stop=True,
                                tile_position=(32 * r, 0),
                            )
                        _exp(i, sc)
                        return
                    # steady-state flavor: 4 chunks x 512 queries; four
                    # K=32 row-group-packed score matmuls run concurrently,
                    # each writing a full DISTINCT psum bank (two [128,1024]
                    # sc tiles); the two exp ops then run on ACT and DVE
                    # concurrently
                    b, u4 = un[1], un[2]
                    fill_pe(FILL_SC)
                    if u4 == 0:
                        av_tiles[b] = av_psum.tile([97, 512], F32, name="av")
                    scA = sc_psum.tile([P, 1024], F32, name="sc")
                    scB = sc_psum.tile([P, 1024], F32, name="sc")
                    q0 = b * 512
                    for r in range(4):
                        cm = 4 * u4 + r
                        rp = 32 * r
                        tgt = scA if r < 2 else scB
                        nc.tensor.matmul(
                            tgt[:, (r % 2) * 512 : (r % 2 + 1) * 512],
                            lhsT=kT[rp : rp + 32, cm * P : (cm + 1) * P],
                            rhs=qT[rp : rp + 32, q0 : q0 + 512],
                            start=True, stop=True,
                            tile_position=(rp, 0),
                        )
                    exA = exp_pool.tile([P, 1024], BF16, name="ex")
                    exB = exp_pool.tile([P, 1024], BF16, name="ex")
                    for half, (sc, ex) in enumerate(((scA, exA), (scB, exB))):
                        if exp_on_act[2 * i + half]:
                            nc.scalar.activation(ex, sc[:], AF.Exp)
                        else:
                            nc.vector.tensor_scalar(
                                ex.bitcast(I16)[:], sc[:], SCH_A, SCH_B,
                                op0=ALU.mult, op1=ALU.add,
                            )
                    ex_tiles[i] = (exA, exB)

                def attn_av(i):
                    un = units[i]
                    ex = ex_tiles.pop(i)
                    if un[0] == "old":
                        b, s = un[1], un[2]
                        if b > 0:
                            fill_pe(FILL_AV)
                        av = av_tiles[b]
                        for r in range(2):
                            cm = 2 * s + r
                            nc.tensor.matmul(
                                av[64 * r : 64 * r + HD + 1, :],
                                lhsT=v_sb[:, cm * (HD + 1) : (cm + 1) * (HD + 1)],
                                rhs=ex[:, r * 512 : (r + 1) * 512],
                                start=(s == 0),
                                stop=(s == 15),
                                tile_position=(0, 64 * r),
                                skip_group_check=True,
                            )
                        return
                    b, u4 = un[1], un[2]
                    fill_pe(FILL_AV)
                    av = av_tiles[b]
                    exA, exB = ex
                    for r in range(4):
                        cm = 4 * u4 + r
                        cp = 64 * (r % 2)
                        exh = exA if r < 2 else exB
                        nc.tensor.matmul(
                            av[cp : cp + HD + 1, :],
                            lhsT=v_sb[:, cm * (HD + 1) : (cm + 1) * (HD + 1)],
                            rhs=exh[:, (r % 2) * 512 : (r % 2 + 1) * 512],
                            start=(u4 == 0 and r < 2),
                            stop=(u4 == 7 and r >= 2),
                            tile_position=(0, cp),
                            skip_group_check=True,
                        )

                epilogue_q = []

                def attn_block_end(b):
                    # queue the evacuate/combine/normalize pieces; one piece
                    # is emitted per subsequent unit so the DVE queue never
                    # bulges and stalls the exp pipeline
                    av = av_tiles.pop(b)
                    q0 = b % 4
                    g = b // 4
                    # DVE TensorTensor may read only one PSUM operand, so ACT
                    # first evacuates the second col-packed half to SBUF
                    tmp33 = work.tile([HD + 1, 512], F32, name="tmp33")
                    epilogue_q.append(lambda: nc.scalar.activation(
                        tmp33, av[64 : 64 + HD + 1, :], AF.Identity))
                    epilogue_q.append(lambda: nc.vector.tensor_tensor(
                        numer4[32 * q0 : 32 * q0 + HD, g, :],
                        av[0:HD, :], tmp33[0:HD, :], ALU.add))
                    epilogue_q.append(lambda: nc.vector.tensor_tensor(
                        den4[32 * q0 : 32 * q0 + 1, g, :],
                        av[HD : HD + 1, :], tmp33[HD : HD + 1, :], ALU.add))
                    if b % 4 == 3:
                        # normalize this group of 4 blocks; quake reciprocal
                        # seed + 1 Newton step stays DVE-only and cheap
                        dg = den4[:, g, :]
                        rg = recip4[:, g, :]
                        bg = bcast4[:, g, :]
                        tg = t512[:, g, :]
                        if USE_RECIP_FAST:
                            epilogue_q.append(
                                lambda: nc.vector.reciprocal_approx_fast(rg, dg))
                        else:
                            epilogue_q.append(lambda: nc.vector.tensor_scalar(
                                rg.bitcast(I32), dg.bitcast(I32), 0, -1,
                                op0=ALU.arith_shift_right, op1=ALU.bitwise_xor))
                            epilogue_q.append(lambda: nc.vector.tensor_scalar_add(
                                rg.bitcast(I32), rg.bitcast(I32), RECIP_MAGIC + 1))
                            epilogue_q.append(lambda: nc.vector.tensor_tensor(
                                tg, dg, rg, ALU.mult))
                            epilogue_q.append(lambda: nc.vector.tensor_scalar(
                                tg, tg, -1.0, 2.0, op0=ALU.mult, op1=ALU.add))
                            epilogue_q.append(lambda: nc.vector.tensor_tensor(
                                rg, rg, tg, ALU.mult))
                        epilogue_q.append(lambda: nc.vector.stream_shuffle(
                            bg, rg, mask=[0] * 32))
                        # group 0's final normalize runs on the (otherwise
                        # idle) Pool engine, far from the critical tail
                        eng = nc.gpsimd if (g == 0 and USE_POOL_NORM) else nc.vector
                        epilogue_q.append(lambda eng=eng: eng.tensor_tensor(
                            out4[:, g, :], numer4[:, g, :], bg, ALU.mult))
                        epilogue_q.append(lambda eng=eng: eng.tensor_scalar_add(
                            out4[:, g, :], out4[:, g, :], bv4))
                        for j in range(4 * g, 4 * g + 4):
                            epilogue_q.append(
                                lambda j=j: (nc.sync if j % 2 == 0 else nc.scalar)
                                .dma_start(
                                    a2a_in[j],
                                    out4[32 * (j % 4) : 32 * (j % 4) + HD, j // 4, :],
                                ))

                def qkv_rank(r):
                    for cc in range(CCH):
                        nc.sync.dma_start(
                            y_full[:, cc, r * NS : (r + 1) * NS],
                            ago[:, cc, r, 0:NS],
                        )
                    # v for the rank's 4 m-chunks
                    for cl in range(4):
                        mb = 4 * r + cl
                        psv = mm_psum.tile([P, 512], F32, name="ps_mm")
                        for cc in range(CCH):
                            nc.tensor.matmul(
                                psv[:, 0:HD],
                                lhsT=y_full[:, cc, mb * P : (mb + 1) * P],
                                rhs=wv_s[:, cc],
                                start=(cc == 0), stop=(cc == CCH - 1),
                            )
                        if cl % 2 == 0:
                            nc.vector.tensor_copy(
                                v_sb[:, mb * (HD + 1) : mb * (HD + 1) + HD],
                                psv[:, 0:HD],
                            )
                        else:
                            nc.scalar.activation(
                                v_sb[:, mb * (HD + 1) : mb * (HD + 1) + HD],
                                psv[:, 0:HD], AF.Identity,
                            )
                    psq = mm_psum.tile([P, 512], F32, name="ps_mm")
                    for cc in range(CCH):
                        nc.tensor.matmul(
                            psq, lhsT=wq_s[:, cc],
                            rhs=y_full[:, cc, r * 512 : (r + 1) * 512],
                            start=(cc == 0), stop=(cc == CCH - 1),
                        )
                    nc.scalar.activation(
                        qT[:, r * 512 : (r + 1) * 512], psq, AF.Identity,
                        bias=bq_eff, scale=SCALE,
                    )
                    psk = mm_psum.tile([P, 512], F32, name="ps_mm")
                    for cc in range(CCH):
                        nc.tensor.matmul(
                            psk, lhsT=wk_s[:, cc],
                            rhs=y_full[:, cc, r * 512 : (r + 1) * 512],
                            start=(cc == 0), stop=(cc == CCH - 1),
                        )
                    nc.vector.tensor_copy(kT[:, r * 512 : (r + 1) * 512], psk)

                # unit stream: block 0 uses the baseline (b, s) shape so the
                # qkv ramp pairing holds (unit s needs rank s//2's kT/qT);
                # blocks 1-7 use (b, h, u) with h-outer so each 256-col AV
                # strip's psum accumulation completes before the next strip
                # starts (CoreSim's pending-zero regions are 2KB-granular).
                # scores run two units ahead of AVs so both exp engines stay
                # saturated; filler matmuls pad the PE's exp-wait gaps.
                if ALL_OLD:
                    units = [("old", b, s) for b in range(NNB)
                             for s in range(16)]
                else:
                    units = [("old", 0, s) for s in range(16)] + [
                        ("big", b, u4) for b in range(1, NNB)
                        for u4 in range(8)
                    ]
                exp_on_act = _exp_schedule(units)

                def prework(i):
                    un = units[i]
                    if un[0] == "old" and un[1] == 0 and un[2] % 2 == 0:
                        qkv_rank(un[2] // 2)
                    if un[1] == 1 and un[2] == 0:
                        # qkv ramp done: park a dead mm_psum tile for fillers
                        state["junk_ps"] = mm_psum.tile([P, 512], F32, name="ps_mm")

                def is_block_last(un):
                    return (un[2] == 15) if un[0] == "old" else (un[2] == 7)

                jptr = [0]

                def emit_scores_upto(i):
                    # keep scores 2 small-units / 1 big-unit ahead of the AVs
                    while jptr[0] < len(units) and jptr[0] - i <= (
                            2 if units[jptr[0]][0] == "old" else 1):
                        prework(jptr[0])
                        attn_score(jptr[0])
                        jptr[0] += 1

                emit_scores_upto(0)
                for i, un in enumerate(units):
                    emit_scores_upto(i + 1)
                    attn_av(i)
                    if is_block_last(un):
                        attn_block_end(un[1])
                    if epilogue_q:
                        epilogue_q.pop(0)()
                while epilogue_q:
                    epilogue_q.pop(0)()

            # ---- phase 5: all-to-all + projection ----
            nc.gpsimd.collective_compute(
                "AllToAll",
                ALU.bypass,
                replica_groups=[list(range(NCORES))],
                ins=[a2a_in[:].opt()],
                outs=[a2a_out[:].opt()],
            )
            cat = a2a_out[:].rearrange("h d f -> (h d) f")  # [256, 512]
            c_sb = persist.tile([P, CCH, NS], BF16, name="c_sb")
            for cc in range(CCH):
                (nc.sync if cc == 0 else nc.scalar).dma_start(
                    c_sb[:, cc], cat[cc * P : (cc + 1) * P]
                )
            out_sb = persist.tile([P, CCH, NS], F32, name="out_sb")
            with tc.tile_pool(name="proj_psum", bufs=2, space="PSUM") as proj_psum:
                for ob in range(CCH):
                    psp = proj_psum.tile([P, NS], F32, name="ps_proj")
                    for cc in range(CCH):
                        nc.tensor.matmul(
                            psp,
                            lhsT=wproj_sb[:, cc, ob * P : (ob + 1) * P],
                            rhs=c_sb[:, cc],
                            start=(cc == 0), stop=(cc == CCH - 1),
                        )
                    nc.scalar.activation(
                        out_sb[:, ob], psp, AF.Identity,
                        bias=bproj_sb[:, ob : ob + 1], scale=1.0,
                    )
                    (nc.sync if ob == 0 else nc.scalar).dma_start(
                        out_d[ob], out_sb[:, ob])

    nc.compile()
    return nc


def _host_prep(x, w_dw, b_dw, w_qkv, b_qkv, w_proj, b_proj):
    """Build per-core in_maps from full inputs."""
    x = np.asarray(x, np.float32)
    w_dw = np.asarray(w_dw, np.float32)
    b_dw = np.asarray(b_dw, np.float32)
    w_qkv = np.asarray(w_qkv, np.float32)
    b_qkv = np.asarray(b_qkv, np.float32)
    w_proj = np.asarray(w_proj, np.float32)
    b_proj = np.asarray(b_proj, np.float32)

    xs = x[0]  # [C, 16, 16, 16]
    # diag conv weights: [CCH, 27, P, P]
    dw_diag = np.zeros((CCH, 27, P, P), np.float32)
    for cc in range(CCH):
        for t, (dh, dw_, dd) in enumerate(TAPS):
            np.fill_diagonal(dw_diag[cc, t], w_dw[cc * P : (cc + 1) * P, 0, dh, dw_, dd])
    dw_diag = dw_diag.astype(bf16)
    b_dw_s = b_dw.reshape(CCH, P, 1)

    wproj_t = np.ascontiguousarray(w_proj.T).reshape(CCH, P, C).astype(bf16)
    bproj_s = b_proj.reshape(CCH, P, 1)

    in_maps = []
    for h in range(NCORES):
        # padded x slab: global h rows 2h-1 .. 2h+2, padded w/d
        xp = np.zeros((C, 4, 18, 18), np.float32)
        for hl in range(4):
            hg = 2 * h - 1 + hl
            if 0 <= hg < HWD:
                xp[:, hl, 1:17, 1:17] = xs[:, hg]
        xp = xp.reshape(CCH, P, 4 * 18 * 18).astype(bf16)

        wq_h = w_qkv[h * HD : (h + 1) * HD]  # [32, 256]
        wk_h = w_qkv[C + h * HD : C + (h + 1) * HD]
        wv_h = w_qkv[2 * C + h * HD : 2 * C + (h + 1) * HD]
        wq_rep = np.tile(wq_h.T, (1, 4)).reshape(C, P)  # [256, 128]
        wk_rep = np.tile(wk_h.T, (1, 4)).reshape(C, P)
        in_maps.append({
            "x_pad": xp,
            "dw_diag": dw_diag,
            "b_dw": b_dw_s,
            "wq": wq_rep.reshape(CCH, P, P).astype(bf16),
            "wk": wk_rep.reshape(CCH, P, P).astype(bf16),
            "wv": np.ascontiguousarray(wv_h.T).reshape(CCH, P, HD).astype(bf16),
            "bq": np.tile(b_qkv[h * HD : (h + 1) * HD], 4).reshape(P, 1).astype(np.float32),
            "bv": b_qkv[2 * C + h * HD : 2 * C + (h + 1) * HD].reshape(HD, 1).astype(np.float32),
            "wproj": wproj_t,
            "bproj": bproj_s,
        })
    return in_maps


def kernel(**inputs):
    if "nc" not in _cache:
        _cache["nc"] = _build_graph()
    nc = _cache["nc"]
    in_maps = _host_prep(**inputs)
    res = bass_utils.run_bass_kernel_spmd(nc, in_maps, core_ids=list(range(NCORES)))
    slices = [res.results[j]["out"].reshape(C, NS) for j in range(NCORES)]
    full = np.concatenate(slices, axis=1)  # [256, 4096]
    return full.reshape(1, C, HWD, HWD, HWD).astype(np.float32)


if __name__ == "__main__":
    nc = _build_graph()
    print("graph built + compiled OK")


# revision 23
# speedup vs baseline: 1.4154x; 1.0499x over previous
"""Distributed Bass kernel for nn_Attention_80908593922315 on 8 TRN2 NeuronCores.

Sharding: head-parallel attention (core h owns head h) + spatial-parallel
conv/proj (core j owns flattened-spatial slice [512j, 512j+512)).

Pipeline per core:
  0. PE warm-up dummies (HAM un-throttle) while inputs stream in
  1. depthwise 3x3x3 conv on PE (27 accumulating diag-matmuls over a
     zero-padded local slab), + per-channel partial stats via ACT accum_out
  2. AllGather of (y, stats) -> full unnormalized y + global instance-norm
     stats; normalization is folded into the QKV weights/biases
  3. per-rank pipeline: y slab DMA -> v/q/k matmuls interleaved with block-0
     attention units; dummy burst gated on the stats DMA re-warms the PE
  4. attention: per unit (4 key-chunks x 256-query strip) scores via FOUR
     K=32 row-group-packed matmuls (concurrent on PE sub-arrays) -> exp
     split ACT (table exp) / DVE (Schraudolph int16 bit-trick), statically
     load-balanced -> AV accumulation [97, 256-strip] x2 col-packed halves
     (row 32 = sum of exp = denominator); small filler matmuls pad PE gaps
     so the HAM clock gate stays at 8/8
  5. per-group normalize (single-op DVE reciprocal + partition broadcast),
     AllToAll (head-major -> spatial slices), 1x1 proj on the local 512
     columns. k-bias is dropped entirely (softmax shift-invariance).
"""

import sys

if "/opt/trn_rl_repo" not in sys.path:
    sys.path.insert(0, "/opt/trn_rl_repo")

import numpy as np
import ml_dtypes

import concourse.bass as bass
import concourse.bacc as bacc
import concourse.tile as tile
import concourse.mybir as mybir
from concourse import bass_utils

BF16 = mybir.dt.bfloat16
F32 = mybir.dt.float32
I16 = mybir.dt.int16
I32 = mybir.dt.int32
AF = mybir.ActivationFunctionType
ALU = mybir.AluOpType
bf16 = ml_dtypes.bfloat16

NCORES = 8
C = 256
NH = 8
HD = 32
HWD = 16
N = HWD * HWD * HWD  # 4096
NS = N // NCORES  # 512 spatial per core
P = 128
CCH = C // P  # 2 channel chunks
EPS = 1e-5
SCALE = HD ** -0.5
NMB = N // P  # 32 m-chunks
NNB = N // 512  # 8 n-blocks
TAPS = [(a, b, c) for a in range(3) for b in range(3) for c in range(3)]

# Schraudolph bf16 exp: exp(x) ~= bitcast_bf16(int16(A*x + B))
SCH_A = 128.0 / float(np.log(2.0))
SCH_B = 16255.5

# PE filler matmuls per unit (before scores / before AV) to keep the HAM
# clock gate at K=8/8 during exp-bound stretches; [1, FILL_W] each
import os
FILL_SC = int(os.environ.get("K_FILL_SC", "0"))
FILL_AV = int(os.environ.get("K_FILL_AV", "2"))
FILL_W = int(os.environ.get("K_FILL_W", "512"))
USE_POOL_NORM = os.environ.get("K_POOL_NORM", "0") == "1"
USE_RECIP_FAST = os.environ.get("K_RECIP_FAST", "0") == "1"
# quake reciprocal seed: r0 = bitcast(MAGIC - i) = bitcast((i ^ -1) + MAGIC+1)
RECIP_MAGIC = 0x7EF311C3
ALL_OLD = os.environ.get("K_ALL_OLD", "1") == "1"
# row-group PE positions for the 4 score matmuls of a new-flavor unit
SC_POS = [int(x) for x in os.environ.get("K_SC_POS", "0,32,64,96").split(",")]

_cache = {}


def _exp_schedule(units):
    """Static greedy ACT/DVE assignment per exp tile (2 slots per unit).

    Models per-tile engine-busy in us: ACT ~0.996, DVE ~1.128, plus the
    side work each engine owes (qkv ramp writes, epilogue).
    """
    A = 0.0
    D = 0.0
    sched = []

    def pick():
        nonlocal A, D
        if A + 0.996 <= D + 1.128:
            sched.append(True)
            A += 0.996
        else:
            sched.append(False)
            D += 1.128

    for un in units:
        if un[0] == "old" and un[1] == 0 and un[2] % 2 == 0:
            A += 0.85  # qT write + v-half
            D += 0.65  # kT write + v-half
        if un[0] == "old":
            pick()
            sched.append(False)  # unused slot
        else:
            pick()
            pick()
        b = un[1]
        is_last = (un[2] == 15) if un[0] == "old" else (un[2] == 7)
        if is_last:
            A += 0.6  # block-end tmp33 evacuate
            D += 1.32  # block-end combine adds
            if b % 4 == 3:
                D += 1.3  # recip + shuffle
                if b // 4 == 1:
                    D += 1.2  # group-1 out4 normalize stays on DVE
    return sched


def _build_graph():
    nc = bacc.Bacc(
        "TRN2",
        target_bir_lowering=False,
        debug=False,
        enable_asserts=True,
        num_devices=NCORES,
    )

    # ---- I/O declarations (per-core shards) ----
    x_pad = nc.dram_tensor("x_pad", [CCH, P, 4 * 18 * 18], BF16, kind="ExternalInput").ap()
    dw_diag = nc.dram_tensor("dw_diag", [CCH, 27, P, P], BF16, kind="ExternalInput").ap()
    b_dw = nc.dram_tensor("b_dw", [CCH, P, 1], F32, kind="ExternalInput").ap()
    wq_d = nc.dram_tensor("wq", [CCH, P, P], BF16, kind="ExternalInput").ap()
    wk_d = nc.dram_tensor("wk", [CCH, P, P], BF16, kind="ExternalInput").ap()
    wv_d = nc.dram_tensor("wv", [CCH, P, HD], BF16, kind="ExternalInput").ap()
    bq_d = nc.dram_tensor("bq", [P, 1], F32, kind="ExternalInput").ap()
    bv_d = nc.dram_tensor("bv", [HD, 1], F32, kind="ExternalInput").ap()
    wproj_d = nc.dram_tensor("wproj", [CCH, P, C], BF16, kind="ExternalInput").ap()
    bproj_d = nc.dram_tensor("bproj", [CCH, P, 1], F32, kind="ExternalInput").ap()
    out_d = nc.dram_tensor("out", [CCH, P, NS], F32, kind="ExternalOutput").ap()

    with tile.TileContext(nc) as tc:
        with tc.tile_pool(name="persist", bufs=1) as persist, \
             tc.tile_pool(name="dram", bufs=1, space="DRAM") as dram, \
             tc.tile_pool(name="work", bufs=4) as work:

            # ---- PE warm-up: keep HAM at K=8/8 from the start so the conv
            # matmuls run at 2.4 GHz; covers the input-DMA window.
            junk = persist.tile([P, P], BF16, name="junk")
            nc.vector.memset(junk[:], 0.0)
            junk2 = persist.tile([P, 512], BF16, name="junk2")
            nc.vector.memset(junk2[:], 0.0)

            # ---- input DMAs, conv-critical first, spread across queues ----
            xp_sb = persist.tile([P, CCH, 4 * 18 * 18], BF16, name="xp_sb")
            dwd_sb = persist.tile([P, CCH, 27, P], BF16, name="dwd_sb")
            # sync queue: chunk-0 conv inputs (first matmul gate)
            nc.sync.dma_start(xp_sb[:, 0], x_pad[0])
            for g in range(4):
                tg = slice(7 * g, min(27, 7 * g + 7))
                nc.sync.dma_start(
                    dwd_sb[:, 0, tg], dw_diag[0, tg].rearrange("t p q -> p t q")
                )
            # scalar queue: chunk-1 conv inputs (ACT is idle this early)
            nc.scalar.dma_start(xp_sb[:, 1], x_pad[1])
            for g in range(4):
                tg = slice(7 * g, min(27, 7 * g + 7))
                nc.scalar.dma_start(
                    dwd_sb[:, 1, tg], dw_diag[1, tg].rearrange("t p q -> p t q")
                )
            bdw_sb = persist.tile([P, CCH], F32, name="bdw_sb")
            for cc in range(CCH):
                nc.scalar.dma_start(bdw_sb[:, cc : cc + 1], b_dw[cc])
            # gpsimd queue: qkv weights (needed right after the AllGather)
            wq_sb = persist.tile([P, CCH, P], BF16, name="wq_sb")
            wk_sb = persist.tile([P, CCH, P], BF16, name="wk_sb")
            wv_sb = persist.tile([P, CCH, HD], BF16, name="wv_sb")
            for cc in range(CCH):
                nc.gpsimd.dma_start(wq_sb[:, cc], wq_d[cc])
                nc.gpsimd.dma_start(wk_sb[:, cc], wk_d[cc])
                nc.gpsimd.dma_start(wv_sb[:, cc], wv_d[cc])
            bq_sb = persist.tile([P, 1], F32, name="bq_sb")
            bv_sb = persist.tile([HD, 1], F32, name="bv_sb")
            nc.gpsimd.dma_start(bq_sb[:], bq_d)
            nc.gpsimd.dma_start(bv_sb[:], bv_d)
            wproj_sb = persist.tile([P, CCH, C], BF16, name="wproj_sb")
            bproj_sb = persist.tile([P, CCH], F32, name="bproj_sb")
            for cc in range(CCH):
                nc.gpsimd.dma_start(wproj_sb[:, cc], wproj_d[cc])
                nc.gpsimd.dma_start(bproj_sb[:, cc : cc + 1], bproj_d[cc])

            # preload the exp activation table early (ACT idle anyway)
            exp_dummy = work.tile([1, 16], BF16, name="exp_dummy")
            nc.vector.memset(exp_dummy[:], 0.0)
            nc.scalar.activation(exp_dummy, exp_dummy, AF.Exp)

            # v ones-columns (softmax denominator rows) set up early
            v_sb = persist.tile([P, NMB * (HD + 1)], BF16, name="v_sb")
            nc.vector.memset(v_sb[:], 1.0)

            # ---- phase 1+2: depthwise conv, then one AllGather of (y, stats)
            y_sb = persist.tile([P, CCH, 514], BF16, name="y_sb")
            stats_sb = persist.tile([P, CCH, 2], F32, name="stats_sb")
            sq_junk = work.tile([P, NS], BF16, name="sq_junk")
            ag_in = dram.tile([P, CCH * 514], BF16, name="ag_in")
            ag_out = dram.tile([NCORES, P, CCH * 514], BF16, name="ag_out",
                               addr_space="Shared")
            with tc.tile_pool(name="warm_psum", bufs=1, space="PSUM") as warm_psum, \
                 tc.tile_pool(name="conv_psum", bufs=2, space="PSUM") as conv_psum:
                wps = warm_psum.tile([1, P], F32, name="wps")
                for _ in range(100):
                    nc.tensor.matmul(wps, lhsT=junk[:, 0:1], rhs=junk[:],
                                     start=True, stop=True)
                for cc in range(CCH):
                    ps = conv_psum.tile([P, NS], F32, name="ps_conv")
                    x4 = xp_sb[:, cc].rearrange("p (a b c) -> p a b c", b=18, c=18)
                    for t, (dh, dw_, dd) in enumerate(TAPS):
                        nc.tensor.matmul(
                            ps,
                            lhsT=dwd_sb[:, cc, t],
                            rhs=x4[:, dh : dh + 2, dw_ : dw_ + 16, dd : dd + 16],
                            start=(t == 0),
                            stop=(t == 26),
                        )
                    nc.scalar.activation(
                        y_sb[:, cc, 0:NS], ps, AF.Identity,
                        bias=bdw_sb[:, cc : cc + 1], scale=1.0,
                        accum_out=stats_sb[:, cc, 0:1],
                    )
                    nc.scalar.activation(
                        sq_junk, ps, AF.Square,
                        bias=bdw_sb[:, cc : cc + 1], scale=1.0,
                        accum_out=stats_sb[:, cc, 1:2],
                    )
                    nc.vector.tensor_copy(y_sb[:, cc, NS : NS + 2], stats_sb[:, cc])
                    # per-chunk bounce, y first (its transfer hides under the
                    # stats chain), then the 2 stats columns gate the trigger
                    q = nc.sync if cc == 0 else nc.scalar
                    q.dma_start(
                        ag_in[:, cc * 514 : cc * 514 + NS], y_sb[:, cc, 0:NS]
                    )
                    q.dma_start(
                        ag_in[:, cc * 514 + NS : (cc + 1) * 514],
                        y_sb[:, cc, NS : NS + 2],
                    )
            nc.gpsimd.collective_compute(
                "AllGather",
                ALU.bypass,
                replica_groups=[list(range(NCORES))],
                ins=[ag_in[:].opt()],
                outs=[ag_out[:].opt()],
            )

            y_full = persist.tile([P, CCH, N], BF16, name="y_full")
            stats_g = work.tile([P, CCH, NCORES, 2], BF16, name="stats_g")
            ssum = work.tile([P, CCH, 2], F32, name="ssum")
            ago = ag_out[:].rearrange("r p (q f) -> p q r f", q=CCH)  # [128,2,8,514]
            # stats first (tiny) so the weight-fold chain runs during the
            # 2MB y_full loads instead of queueing behind them
            for cc in range(CCH):
                nc.sync.dma_start(stats_g[:, cc], ago[:, cc, :, NS : NS + 2])
            for cc in range(CCH):
                nc.vector.reduce_sum(
                    ssum[:, cc],
                    stats_g[:, cc].rearrange("p r s -> p s r"),
                    axis=mybir.AxisListType.X,
                )

            mu = persist.tile([P, CCH], F32, name="mu")
            rstd = persist.tile([P, CCH], F32, name="rstd")
            t_a = work.tile([P, 1], F32, name="t_a")
            t_b = work.tile([P, 1], F32, name="t_b")
            t_c = work.tile([P, 1], F32, name="t_c")
            for cc in range(CCH):
                mcc = mu[:, cc : cc + 1]
                rcc = rstd[:, cc : cc + 1]
                nc.vector.tensor_scalar_mul(mcc, ssum[:, cc, 0:1], 1.0 / N)
                nc.vector.tensor_scalar_mul(t_a, ssum[:, cc, 1:2], 1.0 / N)  # E[y^2]
                nc.vector.tensor_tensor(t_b, mcc, mcc, ALU.mult)  # mu^2
                nc.vector.tensor_tensor(t_a, t_a, t_b, ALU.subtract)  # var
                nc.vector.tensor_scalar_add(t_a, t_a, EPS)  # var+eps
                # quake rsqrt seed (no ACT table set needed) + 2 Newton steps
                rci = rcc.bitcast(I32)
                nc.vector.tensor_scalar(
                    rci, t_a.bitcast(I32), 1, -1,
                    op0=ALU.arith_shift_right, op1=ALU.bitwise_xor,
                )
                nc.vector.tensor_scalar_add(rci, rci, 0x5F3759DF + 1)
                for _ in range(2):
                    nc.vector.tensor_tensor(t_b, rcc, rcc, ALU.mult)
                    nc.vector.tensor_tensor(t_c, t_a, t_b, ALU.mult)
                    nc.vector.tensor_scalar(
                        t_c, t_c, -0.5, 1.5, op0=ALU.mult, op1=ALU.add
                    )
                    nc.vector.tensor_tensor(rcc, rcc, t_c, ALU.mult)

            # fold norm scale into qkv weights
            wq_s = persist.tile([P, CCH, P], BF16, name="wq_s")
            wk_s = persist.tile([P, CCH, P], BF16, name="wk_s")
            wv_s = persist.tile([P, CCH, HD], BF16, name="wv_s")
            mu_bf = work.tile([P, CCH], BF16, name="mu_bf")
            nc.vector.tensor_copy(mu_bf[:], mu[:])
            for cc in range(CCH):
                nc.vector.tensor_scalar_mul(wv_s[:, cc], wv_sb[:, cc], rstd[:, cc : cc + 1])
                nc.vector.tensor_scalar_mul(wq_s[:, cc], wq_sb[:, cc], rstd[:, cc : cc + 1])
                nc.vector.tensor_scalar_mul(wk_s[:, cc], wk_sb[:, cc], rstd[:, cc : cc + 1])

            # ---- phases 3+4 interleaved: per-rank qkv + attention ----
            qT = persist.tile([P, N], BF16, name="qT")
            kT = persist.tile([P, N], BF16, name="kT")
            numer4 = persist.tile([P, 2, 512], F32, name="numer4")
            den4 = persist.tile([P, 2, 512], F32, name="den4")
            recip4 = persist.tile([P, 2, 512], F32, name="recip4")
            bcast4 = persist.tile([P, 2, 512], F32, name="bcast4")
            t512 = persist.tile([P, 2, 512], F32, name="t512")
            nc.vector.memset(den4[:], 1.0)
            out4 = persist.tile([P, 2, 512], BF16, name="out4")
            a2a_in = dram.tile([NCORES, HD, NS + 1], BF16, name="a2a_in")
            a2a_out = dram.tile([NCORES, HD, NS + 1], BF16, name="a2a_out")
            bq_eff = persist.tile([P, 1], F32, name="bq_eff")
            bv_eff = persist.tile([HD, 1], F32, name="bv_eff")

            # PSUM layout (8 banks): sc_psum 3x[128,1024] + av_psum
            # 2x[97,512]. Three sc buffers let scores(i+1) issue while both
            # exp ops of unit i are still reading, killing the PE<->exp
            # ping-pong; the qkv ramp borrows sc tiles for its psum.
            with tc.tile_pool(name="sc_psum", bufs=3, space="PSUM") as sc_psum, \
                 tc.tile_pool(name="av_psum", bufs=2, space="PSUM") as av_psum, \
                 tc.tile_pool(name="exp_pool", bufs=6) as exp_pool:

                # re-warm the PE right as the AllGather lands: a contiguous
                # dummy-matmul burst gated on the stats DMA flips the HAM
                # clock gate to 8/8 before the qkv matmuls start
                warm2 = sc_psum.tile([P, 1024], F32, name="sc")
                for _ in range(70):
                    nc.tensor.matmul(warm2[0:1, 0:16], lhsT=junk[:, 0:1],
                                     rhs=stats_g[:, 0].rearrange("p r s -> p (r s)"),
                                     start=True, stop=True)

                # effective biases: b' = b - W_scaled @ mu  (q also * SCALE).
                # k-bias is dropped: it adds a per-query constant to every
                # logit, which softmax shift-invariance cancels exactly.
                pqv = sc_psum.tile([P, 1024], F32, name="sc")
                pq = pqv[:, 0:1]
                pv = pqv[:, 512:513]
                for cc in range(CCH):
                    nc.tensor.matmul(pq, lhsT=wq_s[:, cc],
                                     rhs=mu_bf[:, cc : cc + 1],
                                     start=(cc == 0), stop=(cc == CCH - 1))
                for cc in range(CCH):
                    nc.tensor.matmul(pv[0:HD], lhsT=wv_s[:, cc],
                                     rhs=mu_bf[:, cc : cc + 1],
                                     start=(cc == 0), stop=(cc == CCH - 1))
                nc.vector.tensor_tensor(bq_eff, bq_sb, pq, ALU.subtract)
                nc.vector.tensor_scalar_mul(bq_eff, bq_eff, SCALE)
                nc.vector.tensor_tensor(bv_eff, bv_sb, pv[0:HD], ALU.subtract)
                bv_bf = persist.tile([HD, 1], BF16, name="bv_bf")
                nc.vector.tensor_copy(bv_bf[:], bv_eff)
                # the v-bias correction rides along the AllToAll (column NS of
                # each a2a slice) and folds into the proj bias afterwards
                for j in range(NCORES):
                    nc.sync.dma_start(a2a_in[j, :, NS : NS + 1], bv_bf[:])

                av_tiles = {}
                ex_tiles = {}
                state = {"av_dead": None}

                def fill_pe(n):
                    # filler matmuls that keep the PE activity monitor fed
                    # while the exp engines catch up; they land in row 0 of
                    # the PREVIOUS block's av tile (already drained by the
                    # epilogue combines, next reused two blocks later)
                    jp = state["av_dead"]
                    if jp is None:
                        return
                    for _ in range(n):
                        nc.tensor.matmul(
                            jp[0:1, 0:FILL_W], lhsT=junk[:, 0:1],
                            rhs=junk2[:, 0:FILL_W], start=True, stop=True,
                            skip_group_check=True,
                        )

                def _exp(i, sc):
                    ex = exp_pool.tile([P, 1024], BF16, name="ex")
                    if exp_on_act[2 * i]:
                        nc.scalar.activation(ex, sc[:], AF.Exp)
                    else:
                        # Schraudolph bf16 exp on DVE: one fused mul-add with
                        # int16 convert-on-write, bit-viewed as bf16
                        nc.vector.tensor_scalar(
                            ex.bitcast(I16)[:], sc[:], SCH_A, SCH_B,
                            op0=ALU.mult, op1=ALU.add,
                        )
                    ex_tiles[i] = ex

                def attn_score(i):
                    un = units[i]
                    if un[0] == "old":
                        # block-0 flavor: 2 chunks x 512 queries (2-way row
                        # packing), interleaves cleanly with the qkv ramp
                        b, s = un[1], un[2]
                        if s == 0:
                            state["av_dead"] = None
                            av_tiles[b] = av_psum.tile([97, 512], F32, name="av")
                        if b > 0:
                            fill_pe(FILL_SC)
                        sc = sc_psum.tile([P, 1024], F32, name="sc")
                        for r in range(2):
                            cm = 2 * s + r
                            nc.tensor.matmul(
                                sc[:, r * 512 : (r + 1) * 512],
                                lhsT=kT[32 * r : 32 * (r + 1), cm * P : (cm + 1) * P],
                                rhs=qT[32 * r : 32 * (r + 1), b * 512 : (b + 1) * 512],
                                start=True, stop=True,
                                tile_position=(32 * r, 0),
                            )
                        _exp(i, sc)
                        return
                    # steady-state flavor: 4 chunks x 512 queries; four
                    # K=32 row-group-packed score matmuls run concurrently,
                    # each writing a full DISTINCT psum bank (two [128,1024]
                    # sc tiles); the two exp ops then run on ACT and DVE
                    # concurrently
                    b, u4 = un[1], un[2]
                    if u4 == 0:
                        state["av_dead"] = None
                        av_tiles[b] = av_psum.tile([97, 512], F32, name="av")
                    fill_pe(FILL_SC)
                    scA = sc_psum.tile([P, 1024], F32, name="sc")
                    scB = sc_psum.tile([P, 1024], F32, name="sc")
                    q0 = b * 512
                    for r in range(4):
                        cm = 4 * u4 + r
                        rp = 32 * r
                        tgt = scA if r < 2 else scB
                        nc.tensor.matmul(
                            tgt[:, (r % 2) * 512 : (r % 2 + 1) * 512],
                            lhsT=kT[rp : rp + 32, cm * P : (cm + 1) * P],
                            rhs=qT[rp : rp + 32, q0 : q0 + 512],
                            start=True, stop=True,
                            tile_position=(rp, 0),
                        )
                    exA = exp_pool.tile([P, 1024], BF16, name="ex")
                    exB = exp_pool.tile([P, 1024], BF16, name="ex")
                    for half, (sc, ex) in enumerate(((scA, exA), (scB, exB))):
                        if exp_on_act[2 * i + half]:
                            nc.scalar.activation(ex, sc[:], AF.Exp)
                        else:
                            nc.vector.tensor_scalar(
                                ex.bitcast(I16)[:], sc[:], SCH_A, SCH_B,
                                op0=ALU.mult, op1=ALU.add,
                            )
                    ex_tiles[i] = (exA, exB)

                def attn_av(i):
                    un = units[i]
                    ex = ex_tiles.pop(i)
                    if un[0] == "old":
                        b, s = un[1], un[2]
                        if b > 0:
                            fill_pe(FILL_AV)
                        av = av_tiles[b]
                        for r in range(2):
                            cm = 2 * s + r
                            nc.tensor.matmul(
                                av[64 * r : 64 * r + HD + 1, :],
                                lhsT=v_sb[:, cm * (HD + 1) : (cm + 1) * (HD + 1)],
                                rhs=ex[:, r * 512 : (r + 1) * 512],
                                start=(s == 0),
                                stop=(s == 15),
                                tile_position=(0, 64 * r),
                                skip_group_check=True,
                            )
                        return
                    b, u4 = un[1], un[2]
                    fill_pe(FILL_AV)
                    av = av_tiles[b]
                    exA, exB = ex
                    for r in range(4):
                        cm = 4 * u4 + r
                        cp = 64 * (r % 2)
                        exh = exA if r < 2 else exB
                        nc.tensor.matmul(
                            av[cp : cp + HD + 1, :],
                            lhsT=v_sb[:, cm * (HD + 1) : (cm + 1) * (HD + 1)],
                            rhs=exh[:, (r % 2) * 512 : (r % 2 + 1) * 512],
                            start=(u4 == 0 and r < 2),
                            stop=(u4 == 7 and r >= 2),
                            tile_position=(0, cp),
                            skip_group_check=True,
                        )

                epilogue_q = []

                def attn_block_end(b):
                    # queue the evacuate/combine/normalize pieces; one piece
                    # is emitted per subsequent unit so the DVE queue never
                    # bulges and stalls the exp pipeline
                    av = av_tiles.pop(b)
                    q0 = b % 4
                    g = b // 4
                    # DVE TensorTensor may read only one PSUM operand, so ACT
                    # first evacuates the second col-packed half to SBUF
                    tmp33 = work.tile([HD + 1, 512], F32, name="tmp33")
                    epilogue_q.append(lambda: nc.scalar.activation(
                        tmp33, av[64 : 64 + HD + 1, :], AF.Identity))
                    epilogue_q.append(lambda: nc.vector.tensor_tensor(
                        numer4[32 * q0 : 32 * q0 + HD, g, :],
                        av[0:HD, :], tmp33[0:HD, :], ALU.add))
                    epilogue_q.append(lambda: nc.vector.tensor_tensor(
                        den4[32 * q0 : 32 * q0 + 1, g, :],
                        av[HD : HD + 1, :], tmp33[HD : HD + 1, :], ALU.add))
                    # av is fully drained once the pieces above are emitted;
                    # from then on its row 0 may serve as the filler target
                    epilogue_q.append(lambda: state.update(av_dead=av))
                    if b % 4 == 3:
                        # normalize this group of 4 blocks; quake reciprocal
                        # seed + 1 Newton step stays DVE-only and cheap
                        dg = den4[:, g, :]
                        rg = recip4[:, g, :]
                        bg = bcast4[:, g, :]
                        tg = t512[:, g, :]
                        if USE_RECIP_FAST:
                            epilogue_q.append(
                                lambda: nc.vector.reciprocal_approx_fast(rg, dg))
                        else:
                            epilogue_q.append(lambda: nc.vector.tensor_scalar(
                                rg.bitcast(I32), dg.bitcast(I32), 0, -1,
                                op0=ALU.arith_shift_right, op1=ALU.bitwise_xor))
                            epilogue_q.append(lambda: nc.vector.tensor_scalar_add(
                                rg.bitcast(I32), rg.bitcast(I32), RECIP_MAGIC + 1))
                            epilogue_q.append(lambda: nc.vector.tensor_tensor(
                                tg, dg, rg, ALU.mult))
                            epilogue_q.append(lambda: nc.vector.tensor_scalar(
                                tg, tg, -1.0, 2.0, op0=ALU.mult, op1=ALU.add))
                            epilogue_q.append(lambda: nc.vector.tensor_tensor(
                                rg, rg, tg, ALU.mult))
                        epilogue_q.append(lambda: nc.vector.stream_shuffle(
                            bg, rg, mask=[0] * 32))
                        # group 0's final normalize runs on the (otherwise
                        # idle) Pool engine, far from the critical tail
                        eng = nc.gpsimd if (g == 0 and USE_POOL_NORM) else nc.vector
                        epilogue_q.append(lambda eng=eng: eng.tensor_tensor(
                            out4[:, g, :], numer4[:, g, :], bg, ALU.mult))
                        for j in range(4 * g, 4 * g + 4):
                            epilogue_q.append(
                                lambda j=j: (nc.sync if j % 2 == 0 else nc.scalar)
                                .dma_start(
                                    a2a_in[j, :, 0:NS],
                                    out4[32 * (j % 4) : 32 * (j % 4) + HD, j // 4, :],
                                ))

                def qkv_rank(r):
                    for cc in range(CCH):
                        nc.sync.dma_start(
                            y_full[:, cc, r * NS : (r + 1) * NS],
                            ago[:, cc, r, 0:NS],
                        )
                    # v for the rank's 4 m-chunks, packed into one sc tile
                    psv = sc_psum.tile([P, 1024], F32, name="sc")
                    for cl in range(4):
                        mb = 4 * r + cl
                        pv_ = psv[:, 256 * cl : 256 * cl + HD]
                        for cc in range(CCH):
                            nc.tensor.matmul(
                                pv_,
                                lhsT=y_full[:, cc, mb * P : (mb + 1) * P],
                                rhs=wv_s[:, cc],
                                start=(cc == 0), stop=(cc == CCH - 1),
                            )
                        if cl % 2 == 0:
                            nc.vector.tensor_copy(
                                v_sb[:, mb * (HD + 1) : mb * (HD + 1) + HD],
                                pv_,
                            )
                        else:
                            nc.scalar.activation(
                                v_sb[:, mb * (HD + 1) : mb * (HD + 1) + HD],
                                pv_, AF.Identity,
                            )
                    psqk = sc_psum.tile([P, 1024], F32, name="sc")
                    psq = psqk[:, 0:512]
                    psk = psqk[:, 512:1024]
                    for cc in range(CCH):
                        nc.tensor.matmul(
                            psq, lhsT=wq_s[:, cc],
                            rhs=y_full[:, cc, r * 512 : (r + 1) * 512],
                            start=(cc == 0), stop=(cc == CCH - 1),
                        )
                    nc.scalar.activation(
                        qT[:, r * 512 : (r + 1) * 512], psq, AF.Identity,
                        bias=bq_eff, scale=SCALE,
                    )
                    for cc in range(CCH):
                        nc.tensor.matmul(
                            psk, lhsT=wk_s[:, cc],
                            rhs=y_full[:, cc, r * 512 : (r + 1) * 512],
                            start=(cc == 0), stop=(cc == CCH - 1),
                        )
                    nc.vector.tensor_copy(kT[:, r * 512 : (r + 1) * 512], psk)

                # unit stream: block 0 uses the baseline (b, s) shape so the
                # qkv ramp pairing holds (unit s needs rank s//2's kT/qT);
                # blocks 1-7 use (b, h, u) with h-outer so each 256-col AV
                # strip's psum accumulation completes before the next strip
                # starts (CoreSim's pending-zero regions are 2KB-granular).
                # scores run two units ahead of AVs so both exp engines stay
                # saturated; filler matmuls pad the PE's exp-wait gaps.
                if ALL_OLD:
                    units = [("old", b, s) for b in range(NNB)
                             for s in range(16)]
                else:
                    units = [("old", 0, s) for s in range(16)] + [
                        ("big", b, u4) for b in range(1, NNB)
                        for u4 in range(8)
                    ]
                exp_on_act = _exp_schedule(units)

                def prework(i):
                    un = units[i]
                    if un[0] == "old" and un[1] == 0 and un[2] % 2 == 0:
                        qkv_rank(un[2] // 2)


                def is_block_last(un):
                    return (un[2] == 15) if un[0] == "old" else (un[2] == 7)

                jptr = [0]

                def emit_scores_upto(i):
                    # keep scores 2 small-units / 1 big-unit ahead of the AVs
                    while jptr[0] < len(units) and jptr[0] - i <= (
                            2 if units[jptr[0]][0] == "old" else 1):
                        prework(jptr[0])
                        attn_score(jptr[0])
                        jptr[0] += 1

                emit_scores_upto(0)
                for i, un in enumerate(units):
                    emit_scores_upto(i + 1)
                    attn_av(i)
                    if is_block_last(un):
                        attn_block_end(un[1])
                    if epilogue_q:
                        epilogue_q.pop(0)()
                while epilogue_q:
                    epilogue_q.pop(0)()

            # ---- phase 5: all-to-all + projection ----
            nc.gpsimd.collective_compute(
                "AllToAll",
                ALU.bypass,
                replica_groups=[list(range(NCORES))],
                ins=[a2a_in[:].opt()],
                outs=[a2a_out[:].opt()],
            )
            cat = a2a_out[:].rearrange("h d f -> (h d) f")  # [256, 513]
            c_sb = persist.tile([P, CCH, NS], BF16, name="c_sb")
            bvec_sb = persist.tile([P, CCH], BF16, name="bvec_sb")
            for cc in range(CCH):
                (nc.sync if cc == 0 else nc.scalar).dma_start(
                    c_sb[:, cc], cat[cc * P : (cc + 1) * P, 0:NS]
                )
                nc.sync.dma_start(
                    bvec_sb[:, cc : cc + 1], cat[cc * P : (cc + 1) * P, NS : NS + 1]
                )
            out_sb = persist.tile([P, CCH, NS], F32, name="out_sb")
            bp_eff = persist.tile([P, CCH], F32, name="bp_eff")
            with tc.tile_pool(name="proj_psum", bufs=2, space="PSUM") as proj_psum:
                # bias fold: bp_eff[:, ob] = bproj + W_proj[ob-chunk] @ bvec
                psb = proj_psum.tile([P, NS], F32, name="ps_proj")
                for ob in range(CCH):
                    for cc in range(CCH):
                        nc.tensor.matmul(
                            psb[:, ob : ob + 1],
                            lhsT=wproj_sb[:, cc, ob * P : (ob + 1) * P],
                            rhs=bvec_sb[:, cc : cc + 1],
                            start=(cc == 0), stop=(cc == CCH - 1),
                        )
                for ob in range(CCH):
                    nc.vector.tensor_tensor(
                        bp_eff[:, ob : ob + 1], bproj_sb[:, ob : ob + 1],
                        psb[:, ob : ob + 1], ALU.add,
                    )
                for ob in range(CCH):
                    psp = proj_psum.tile([P, NS], F32, name="ps_proj")
                    for cc in range(CCH):
                        nc.tensor.matmul(
                            psp,
                            lhsT=wproj_sb[:, cc, ob * P : (ob + 1) * P],
                            rhs=c_sb[:, cc],
                            start=(cc == 0), stop=(cc == CCH - 1),
                        )
                    nc.scalar.activation(
                        out_sb[:, ob], psp, AF.Identity,
                        bias=bp_eff[:, ob : ob + 1], scale=1.0,
                    )
                    (nc.sync if ob == 0 else nc.scalar).dma_start(
                        out_d[ob], out_sb[:, ob])

    nc.compile()
    return nc


def _host_prep(x, w_dw, b_dw, w_qkv, b_qkv, w_proj, b_proj):
    """Build per-core in_maps from full inputs."""
    x = np.asarray(x, np.float32)
    w_dw = np.asarray(w_dw, np.float32)
    b_dw = np.asarray(b_dw, np.float32)
    w_qkv = np.asarray(w_qkv, np.float32)
    b_qkv = np.asarray(b_qkv, np.float32)
    w_proj = np.asarray(w_proj, np.float32)
    b_proj = np.asarray(b_proj, np.float32)

    xs = x[0]  # [C, 16, 16, 16]
    # diag conv weights: [CCH, 27, P, P]
    dw_diag = np.zeros((CCH, 27, P, P), np.float32)
    for cc in range(CCH):
        for t, (dh, dw_, dd) in enumerate(TAPS):
            np.fill_diagonal(dw_diag[cc, t], w_dw[cc * P : (cc + 1) * P, 0, dh, dw_, dd])
    dw_diag = dw_diag.astype(bf16)
    b_dw_s = b_dw.reshape(CCH, P, 1)

    wproj_t = np.ascontiguousarray(w_proj.T).reshape(CCH, P, C).astype(bf16)
    bproj_s = b_proj.reshape(CCH, P, 1)

    in_maps = []
    for h in range(NCORES):
        # padded x slab: global h rows 2h-1 .. 2h+2, padded w/d
        xp = np.zeros((C, 4, 18, 18), np.float32)
        for hl in range(4):
            hg = 2 * h - 1 + hl
            if 0 <= hg < HWD:
                xp[:, hl, 1:17, 1:17] = xs[:, hg]
        xp = xp.reshape(CCH, P, 4 * 18 * 18).astype(bf16)

        wq_h = w_qkv[h * HD : (h + 1) * HD]  # [32, 256]
        wk_h = w_qkv[C + h * HD : C + (h + 1) * HD]
        wv_h = w_qkv[2 * C + h * HD : 2 * C + (h + 1) * HD]
        wq_rep = np.tile(wq_h.T, (1, 4)).reshape(C, P)  # [256, 128]
        wk_rep = np.tile(wk_h.T, (1, 4)).reshape(C, P)
        in_maps.append({
            "x_pad": xp,
            "dw_diag": dw_diag,
            "b_dw": b_dw_s,
            "wq": wq_rep.reshape(CCH, P, P).astype(bf16),
            "wk": wk_rep.reshape(CCH, P, P).astype(bf16),
            "wv": np.ascontiguousarray(wv_h.T).reshape(CCH, P, HD).astype(bf16),
            "bq": np.tile(b_qkv[h * HD : (h + 1) * HD], 4).reshape(P, 1).astype(np.float32),
            "bv": b_qkv[2 * C + h * HD : 2 * C + (h + 1) * HD].reshape(HD, 1).astype(np.float32),
            "wproj": wproj_t,
            "bproj": bproj_s,
        })
    return in_maps


def kernel(**inputs):
    if "nc" not in _cache:
        _cache["nc"] = _build_graph()
    nc = _cache["nc"]
    in_maps = _host_prep(**inputs)
    res = bass_utils.run_bass_kernel_spmd(nc, in_maps, core_ids=list(range(NCORES)))
    slices = [res.results[j]["out"].reshape(C, NS) for j in range(NCORES)]
    full = np.concatenate(slices, axis=1)  # [256, 4096]
    return full.reshape(1, C, HWD, HWD, HWD).astype(np.float32)


if __name__ == "__main__":
    nc = _build_graph()
    print("graph built + compiled OK")
